# revision 23
# baseline (speedup 1.0000x reference)
"""Trainium2 Bass kernel for nn_Net_90331752170289 (Mamba block + FFT/CNN + fusion head).

Strategy: pure data parallelism over batch (8 batches per core on 8 cores).

v4: phase-C broadcasts (delta/dx/B/C fan-outs) are mask-matmuls into PSUM
(SBUF->SBUF broadcast DMAs cost ~500ns per 4KB per-partition descriptor and
dominated v1).  The DFT keeps x stationary on the PE and streams the DFT
matrix as the moving operand so |FFT| lands as [96 seqs, freq] directly --
no transposes; the mirror half is a reversed-stride vector copy.  Phases
D (out_proj/LN/FFN/LN2) and F (fusion head) are split per batch-group g so
g=0's chain overlaps the second half of the scan phase.  dBx is copied
PSUM->SBUF on the scalar engine so the B-multiply runs in the DVE 2x mode.
hc multiplies ride the otherwise-idle GPSIMD.  Most tensors are bf16.

Lane layout for scan tiles: lane = 8*n + dl  (d = 8*t3 + dl).
X24 tensors (xi, siluz, delta, dx, BC): [128, 2048] per g, row = 32*bi + ch.
X12 tensors (hhat, s_t, xm_hat, xcnn): [64, 2048] per g, row = 16*b' + m.
"""
import numpy as np

B, L, DM = 64, 2048, 12
DI, DS, DC = 24, 16, 4
NCORES = 8
BL = B // NCORES          # 8 local batches per core
N = L                     # free dim per batch
NC2 = 512                 # psum chunk (1 bank)
NF = 1152                 # padded rfft bins (valid 0..1024)
NKT = L // 128            # 16 DFT K-tiles
SQ2I = 0.7071067811865476

_CACHE = {}


# ---------------------------------------------------------------- device code
def _build_module():
    import concourse.bacc as bacc
    import concourse.bass as bass
    import concourse.tile as tile
    from concourse import mybir
    from contextlib import ExitStack

    F32 = mybir.dt.float32
    F32R = mybir.dt.float32r
    F16 = mybir.dt.float16
    BF16 = mybir.dt.bfloat16
    AF = mybir.ActivationFunctionType
    OP = mybir.AluOpType
    AX = mybir.AxisListType

    nc = bacc.Bacc("TRN2", target_bir_lowering=False, debug=False)

    def din(name, shape, dt=F32R):
        return nc.dram_tensor(name, shape, dt, kind="ExternalInput")

    # per-core data
    xs_d = din("xs", [4, 96, N], BF16)            # in_proj rhs, per b-pair
    xt_d = din("xt", [128, NKT * 96], F16)        # DFT lhsT, kt-major cols
    wdft_d = din("wdft", [2, NKT, 128, 1024], F16)  # [fchunk, kt, t1, cs*f]
    nyc_d = din("nyc", [128, NKT], F16)           # (-1)^t cos col per kt
    # folded weights (identical on all cores)
    w_xc_d = din("w_xc", [96, 64], BF16)
    w_z_d = din("w_z", [96, 64], BF16)
    w_delta_d = din("w_delta", [128, 128], BF16)
    w_bc_d = din("w_bc", [128, 128], BF16)
    w_op_d = din("w_op", [128, 64], BF16)
    w_ones12_d = din("w_ones12", [128, 8], BF16)
    w_bc8_d = din("w_bc8", [8, 128])
    w_ffn1_d = din("w_ffn1", [4, 128, 128], BF16)
    w_ffn2_d = din("w_ffn2", [4, 128, 32], BF16)
    w_pc_d = din("w_pc", [128, 128], BF16)
    w_lin1a_d = din("w_lin1a", [128, 128], BF16)
    w_lin1b_d = din("w_lin1b", [128, 128], BF16)
    w_lin2_d = din("w_lin2", [2, 128, 128], BF16)
    w_lin3_d = din("w_lin3", [2, 128, 4], BF16)
    w_cnn_d = din("w_cnn", [3, 96, 128], F16)
    w_mask_d = din("w_mask", [3, 128, 32], BF16)
    masks_d = din("masks", [128, 20 * 128], BF16)  # 12 dbc + 4 B + 4 C
    sc_negA_d = din("sc_negA", [3, 128, 1], F32)
    vec_d = din("vecs", [128, 11], F32)           # packed per-partition vectors
    w_dp_d = din("w_dp", [4, 128, 32], BF16)
    b_out_d = din("b_out", [8, 1], F32)
    b_eps_d = din("b_eps", [8, 1], F32)
    (V_BCONV, V_BDT, V_SDP, V_G1, V_B1, V_BFFN1, V_BFFN2, V_BHEAD1,
     V_BLIN2, V_BCNN, V_BFFN1S) = range(11)

    out_d = nc.dram_tensor("out", [8, 1], F32, kind="ExternalOutput")

    with tile.TileContext(nc) as tc, ExitStack() as ctx:
        sg = ctx.enter_context(tc.tile_pool(name="singles", bufs=1))
        ws = ctx.enter_context(tc.tile_pool(name="work", bufs=2))
        big = ctx.enter_context(tc.tile_pool(name="big", bufs=1))
        cp = ctx.enter_context(tc.tile_pool(name="cpool", bufs=2))
        hcp = ctx.enter_context(tc.tile_pool(name="hcpool", bufs=1))
        wdp = ctx.enter_context(tc.tile_pool(name="wdftpool", bufs=3))
        pmm = ctx.enter_context(tc.tile_pool(name="pmm", bufs=1, space="PSUM"))
        pd = ctx.enter_context(tc.tile_pool(name="pd", bufs=2, space="PSUM"))
        pdft = ctx.enter_context(tc.tile_pool(name="pdft", bufs=1,
                                              space="PSUM"))

        def load(dram_ap, shape, dt, tag, pool=sg):
            t = pool.tile(shape, dt, tag=tag, name=tag)
            nc.sync.dma_start(out=t, in_=dram_ap)
            return t

        def act(out, in_, func, **kw):
            return nc.scalar.activation(out, in_, func, **kw)

        # ---- load weights/constants into SBUF
        W_xc = load(w_xc_d[:, :], [96, 64], BF16, "w_xc")
        W_z = load(w_z_d[:, :], [96, 64], BF16, "w_z")
        W_delta = load(w_delta_d[:, :], [128, 128], BF16, "w_delta")
        W_bc = load(w_bc_d[:, :], [128, 128], BF16, "w_bc")
        W_op = load(w_op_d[:, :], [128, 64], BF16, "w_op")
        W_ones12 = load(w_ones12_d[:, :], [128, 8], BF16, "w_ones12")
        W_bc8 = load(w_bc8_d[:, :], [8, 128], F32R, "w_bc8")
        W_ffn1 = [load(w_ffn1_d[q], [128, 128], BF16, f"w_ffn1_{q}")
                  for q in range(4)]
        W_ffn2 = [load(w_ffn2_d[q], [128, 32], BF16, f"w_ffn2_{q}")
                  for q in range(4)]
        W_pc = load(w_pc_d[:, :], [128, 128], BF16, "w_pc")
        W_lin1a = load(w_lin1a_d[:, :], [128, 128], BF16, "w_lin1a")
        W_lin1b = load(w_lin1b_d[:, :], [128, 128], BF16, "w_lin1b")
        W_lin2 = [load(w_lin2_d[g], [128, 128], BF16, f"w_lin2_{g}")
                  for g in range(2)]
        W_lin3 = [load(w_lin3_d[g], [128, 4], BF16, f"w_lin3_{g}")
                  for g in range(2)]
        W_cnn = [load(w_cnn_d[k], [96, 128], F16, f"w_cnn_{k}")
                 for k in range(3)]
        W_mask = [load(w_mask_d[t], [128, 32], BF16, f"w_mask_{t}")
                  for t in range(3)]
        Masks = sg.tile([128, 20 * 128], BF16, tag="masks", name="masks")
        for i in range(4):
            nc.sync.dma_start(out=Masks[:, 640 * i:640 * i + 640],
                              in_=masks_d[:, 640 * i:640 * i + 640])
        M_dbc = [Masks[:, 128 * i:128 * i + 128] for i in range(12)]
        M_b = [Masks[:, 128 * (12 + i):128 * (12 + i) + 128] for i in range(4)]
        M_c = [Masks[:, 128 * (16 + i):128 * (16 + i) + 128] for i in range(4)]
        ScA = [load(sc_negA_d[t], [128, 1], F32, f"scA_{t}") for t in range(3)]
        Vec_t = load(vec_d[:, :], [128, 11], F32, "vec_t")
        Vec = [Vec_t[:, i:i + 1] for i in range(11)]
        W_dp = [load(w_dp_d[bi], [128, 32], BF16, f"w_dp{bi}")
                for bi in range(4)]
        Bout = load(b_out_d[:, :], [8, 1], F32, "b_out")
        Beps = load(b_eps_d[:, :], [8, 1], F32, "b_eps")

        # ---- persistent activations
        xi = [big.tile([128, N], BF16, tag=f"xi{g}", name=f"xi{g}")
              for g in range(2)]
        siluz = [big.tile([128, N], BF16, tag=f"siluz{g}", name=f"siluz{g}")
                 for g in range(2)]
        delta = [big.tile([128, N], BF16, tag=f"delta{g}", name=f"delta{g}")
                 for g in range(2)]
        dx = [big.tile([128, N], BF16, tag=f"dx{g}", name=f"dx{g}")
              for g in range(2)]
        BC = [big.tile([128, N], BF16, tag=f"bc{g}", name=f"bc{g}")
              for g in range(2)]
        y = [big.tile([128, N], BF16, tag=f"y{g}", name=f"y{g}")
             for g in range(2)]
        xfT = big.tile([96, N + 2], F16, tag="xfT", name="xfT")
        racc = [sg.tile([4, 1], F32, tag=f"racc{g}", name=f"racc{g}")
                for g in range(2)]

        C4 = [c * NC2 for c in range(N // NC2)]

        # ================= phase E: DFT |FFT| (x stationary, W moving) =====
        def _phase_E():
            xt_sb = sg.tile([128, NKT * 96], F16, tag="xt", name="xt")
            nc.sync.dma_start(out=xt_sb, in_=xt_d[:, :])
            nyc = load(nyc_d[:, :], [128, NKT], F16, "nyc")
            nc.vector.memset(xfT[:, 0:1], 0.0)
            nc.vector.memset(xfT[:, N + 1:N + 2], 0.0)
            for ch in range(2):
                f0 = 512 * ch
                pc = pdft.tile([96, 512], F32, tag="pdc", name="pdc")
                ps = pdft.tile([96, 512], F32, tag="pds", name="pds")
                for kt in range(NKT):
                    wsl = wdp.tile([128, 1024], F16, tag="wsl", name="wsl")
                    nc.sync.dma_start(out=wsl, in_=wdft_d[ch, kt])
                    xk = xt_sb[:, 96 * kt:96 * kt + 96]
                    nc.tensor.matmul(pc, xk, wsl[:, 0:512],
                                     start=(kt == 0), stop=(kt == NKT - 1))
                    nc.tensor.matmul(ps, xk, wsl[:, 512:1024],
                                     start=(kt == 0), stop=(kt == NKT - 1))
                sqs = ws.tile([96, 512], F32, tag="sqcs", name="sqcs")
                sq2 = ws.tile([96, 512], F32, tag="sqcs2", name="sqcs2")
                act(sqs, pc, AF.Square)
                act(sq2, ps, AF.Square)
                nc.vector.scalar_tensor_tensor(sqs, sqs, 1e-20, sq2,
                                               OP.add, OP.add)
                lnm = ws.tile([96, 512], F32, tag="lnm", name="lnm")
                act(lnm, sqs, AF.Ln)
                act(xfT[:, 1 + f0:1 + f0 + 512], lnm, AF.Exp, scale=0.5)
                # mirror: f in [f0, f0+512) -> cols 1 + (2048 - f), desc
                nsrc = 511 if ch == 0 else 512
                rev = bass.AP(tensor=xfT.tensor,
                              offset=xfT.offset + (2048 if ch == 0 else 1537),
                              ap=[list(xfT.ap[0]), [-1, nsrc]])
                nc.vector.tensor_copy(
                    rev, xfT[:, 2 + f0:2 + f0 + nsrc] if ch == 0
                    else xfT[:, 1 + f0:1 + f0 + nsrc])
            # Nyquist bin f=1024: X = sum_t x[t] cos(pi t); sin part is 0
            pny = pdft.tile([96, 1], F32, tag="pdc", name="pny")
            for kt in range(NKT):
                nc.tensor.matmul(pny, xt_sb[:, 96 * kt:96 * kt + 96],
                                 nyc[:, kt:kt + 1],
                                 start=(kt == 0), stop=(kt == NKT - 1))
            sqn = ws.tile([96, 1], F32, tag="rc", name="sqn")
            act(sqn, pny, AF.Square)
            lnn = ws.tile([96, 1], F32, tag="rc", name="lnn")
            act(lnn, sqn, AF.Ln)
            act(xfT[:, 1025:1026], lnn, AF.Exp, scale=0.5)

        # ================= phases D + F (all 8 batches, rows 16b+m) =========
        def _phase_DF():
            hhat = big.tile([128, N], BF16, tag="hhat", name="hhat")
            h_aff = big.tile([128, N], BF16, tag="h_aff", name="h_aff")
            CH = [(c * 1024, c * 1024 + 1024) for c in range(N // 1024)]

            def mm512(p, lhsT, rhs, c0, c1, start=True, stop=True):
                for o in range(0, c1 - c0, 512):
                    nc.tensor.matmul(p[:, o:o + 512], lhsT,
                                     rhs[:, c0 + o:c0 + o + 512],
                                     start=start, stop=stop)

            for c0, c1 in CH:
                p_m = [pmm.tile([64, 1024], F32, tag=tg, name="p_m")
                       for tg in ("pdl", "pdx")]
                for g in range(2):
                    mm512(p_m[g], W_op, y[g], c0, c1)
                cent = ws.tile([128, 1024], BF16, tag="cent", name="cent")
                sq = ws.tile([128, 1024], BF16, tag="sq", name="sq")
                for g in range(2):
                    gg = slice(64 * g, 64 * g + 64)
                    act(cent[gg, :], p_m[g], AF.Copy)
                    act(sq[gg, :], p_m[g], AF.Square)
                p_v = pmm.tile([8, 1024], F32, tag="pdl", name="p_v")
                mm512(p_v, W_ones12, sq, 0, 1024)
                sd = ws.tile([8, 1024], F32, tag="sd", name="sd")
                act(sd, p_v, AF.Ln, bias=Beps)
                inv = ws.tile([8, 1024], F32R, tag="sd", name="inv")
                act(inv, sd, AF.Exp, scale=-0.5)
                p_b = pmm.tile([128, 1024], F32, tag="pdx", name="p_b")
                mm512(p_b, W_bc8, inv, 0, 1024)
                nc.vector.tensor_mul(hhat[:, c0:c1], cent, p_b)
                nc.vector.tensor_scalar(h_aff[:, c0:c1], hhat[:, c0:c1],
                                        Vec[V_G1], Vec[V_B1],
                                        OP.mult, OP.add)
            # FFN; gelu=0.5*u*(1+erf(u/sqrt2)), 0.5 folded in W_ffn2
            s_t = big.tile([128, N], BF16, tag="s_t", name="s_t")
            for q in range(4):
                for c0, c1 in CH:
                    p_f = pmm.tile([128, 1024], F32, tag="pdl", name="p_f")
                    mm512(p_f, W_ffn1[q], hhat, c0, c1)
                    u_b = ws.tile([128, 1024], BF16, tag="sgm", name="u_b")
                    act(u_b, p_f, AF.Identity, bias=Vec[V_BFFN1])
                    erf_t = ws.tile([128, 1024], BF16, tag="erf", name="erf")
                    act(erf_t, u_b, AF.Erf, scale=SQ2I)
                    ff_c = ws.tile([128, 1024], BF16, tag="ffch", name="ff_c")
                    nc.vector.scalar_tensor_tensor(
                        ff_c, erf_t, 1.0, u_b, OP.add, OP.mult)
                    p_2 = pmm.tile([32, 1024], F32, tag="pdx", name="p_2")
                    mm512(p_2, W_ffn2[q], ff_c, 0, 1024)
                    rq = slice(32 * q, 32 * q + 32)
                    nc.vector.scalar_tensor_tensor(
                        s_t[rq, c0:c1], p_2, Vec[V_BFFN2][rq, :],
                        h_aff[rq, c0:c1], OP.add, OP.add)
            # LN2
            xm_hat = big.tile([128, N], BF16, tag="xm_hat", name="xm_hat")
            for c0, c1 in CH:
                p_c = pmm.tile([128, 1024], F32, tag="pdl", name="p_c")
                mm512(p_c, W_pc, s_t, c0, c1)
                c2 = ws.tile([128, 1024], BF16, tag="cent", name="c2")
                act(c2, p_c, AF.Copy)
                sq2 = ws.tile([128, 1024], BF16, tag="sq", name="sq2")
                act(sq2, p_c, AF.Square)
                p_v2 = pmm.tile([8, 1024], F32, tag="pdx", name="p_v2")
                mm512(p_v2, W_ones12, sq2, 0, 1024)
                sd2 = ws.tile([8, 1024], F32, tag="sd", name="sd2")
                act(sd2, p_v2, AF.Ln, bias=Beps)
                inv2 = ws.tile([8, 1024], F32R, tag="sd", name="inv2")
                act(inv2, sd2, AF.Exp, scale=-0.5)
                p_b2 = pmm.tile([128, 1024], F32, tag="pdl", name="p_b2")
                mm512(p_b2, W_bc8, inv2, 0, 1024)
                nc.vector.tensor_mul(xm_hat[:, c0:c1], c2, p_b2)
            # CNN (xfT ready early; 3 shifted block-diag matmuls)
            xcnn = big.tile([128, N], BF16, tag="xcnn", name="xcnn")
            for c0, c1 in CH:
                p_cn = pmm.tile([128, 1024], F32, tag="pdx", name="p_cn")
                for k in range(3):
                    mm512(p_cn, W_cnn[k], xfT, c0 + k, c1 + k,
                          start=(k == 0), stop=(k == 2))
                act(xcnn[:, c0:c1], p_cn, AF.Identity, bias=Vec[V_BCNN])
            # fusion head
            for g in range(2):
                nc.vector.memset(racc[g], 0.0)
            for c0, c1 in CH:
                p_1 = pmm.tile([128, 1024], F32, tag="pdl", name="p_1")
                mm512(p_1, W_lin1a, xm_hat, c0, c1, start=True, stop=False)
                mm512(p_1, W_lin1b, xcnn, c0, c1, start=False, stop=True)
                mneg = ws.tile([128, 1024], BF16, tag="mneg", name="mneg")
                nc.vector.tensor_scalar(mneg, p_1, Vec[V_BHEAD1], 0.0,
                                        OP.add, OP.min)
                e_t = ws.tile([128, 1024], BF16, tag="e_t", name="e_t")
                act(e_t, mneg, AF.Exp)
                r_t = ws.tile([128, 1024], BF16, tag="r_t", name="r_t")
                act(r_t, p_1, AF.Relu, bias=Vec[V_BHEAD1])
                v_t = ws.tile([128, 1024], BF16, tag="e_t", name="v_t")
                nc.vector.tensor_add(v_t, r_t, e_t)
                for g in range(2):
                    p_o2 = pmm.tile([128, 1024], F32, tag="pdx", name="p_o2")
                    mm512(p_o2, W_lin2[g], v_t, 0, 1024)
                    o2c = ws.tile([128, 1024], BF16, tag="mneg", name="o2c")
                    act(o2c, p_o2, AF.Identity, bias=Vec[V_BLIN2])
                    p_o3 = pmm.tile([4, 1024], F32, tag="pdl", name="p_o3")
                    mm512(p_o3, W_lin3[g], o2c, 0, 1024)
                    o3c = ws.tile([4, 1024], F32, tag="sd", name="o3c")
                    act(o3c, p_o3, AF.Copy)
                    rc = ws.tile([4, 1], F32, tag="rc", name="rc")
                    nc.vector.tensor_reduce(rc, o3c, AX.X, OP.add)
                    nc.vector.tensor_add(racc[g], racc[g], rc)
            for g in range(2):
                res = sg.tile([4, 1], F32, tag=f"res{g}", name=f"res{g}")
                act(res, racc[g], AF.Sigmoid, bias=Bout[0:4, :],
                    scale=1.0 / N)
                nc.sync.dma_start(out=out_d[4 * g:4 * g + 4, :], in_=res)

        # ================= phase A: fused in_proj + causal conv, silu =======
        for g in range(2):
            xsp = [ws.tile([96, N], BF16, tag="pairA", name="xsp")
                   for j in range(2)]
            for j in range(2):
                for ph in range(2):
                    nc.sync.dma_start(out=xsp[j][48 * ph:48 * ph + 48, :],
                                      in_=xs_d[2 * g + j, 48 * ph:48 * ph + 48])
            for c0 in C4:
                sl = slice(c0, c0 + NC2)
                for j in range(2):
                    jj = slice(64 * j, 64 * j + 64)
                    p_xc = pmm.tile([64, 1024], F32, tag="pdl", name="pmm")
                    nc.tensor.matmul(p_xc[:, 0:NC2], W_xc, xsp[j][:, sl])
                    act(xi[g][jj, sl], p_xc[:, 0:NC2], AF.Silu,
                        bias=Vec[V_BCONV][jj, :])
            for c0 in C4:
                sl = slice(c0, c0 + NC2)
                for j in range(2):
                    jj = slice(64 * j, 64 * j + 64)
                    p_z = pmm.tile([64, 1024], F32, tag="pdx", name="pmm")
                    nc.tensor.matmul(p_z[:, 0:NC2], W_z, xsp[j][:, sl])
                    act(siluz[g][jj, sl], p_z[:, 0:NC2], AF.Silu)

        # ================= phase B: x_proj (delta folded), dx ==============
        for g in range(2):
            for c0 in C4:
                sl = slice(c0, c0 + NC2)
                p_d = pmm.tile([128, 1024], F32, tag="pdl", name="pmm")
                nc.tensor.matmul(p_d[:, 0:NC2], W_delta, xi[g][:, sl])
                edt = ws.tile([128, NC2], F32, tag="sgm", name="edt")
                act(edt, p_d[:, 0:NC2], AF.Exp, bias=Vec[V_BDT])
                act(delta[g][:, sl], edt, AF.Ln, bias=1.0)
                p_bc = pmm.tile([128, 1024], F32, tag="pdx", name="pmm")
                nc.tensor.matmul(p_bc[:, 0:NC2], W_bc, xi[g][:, sl])
                act(BC[g][:, sl], p_bc[:, 0:NC2], AF.Copy)
            nc.vector.tensor_mul(dx[g], delta[g], xi[g])

        # ================= phase C: selective scan ==========================
        for b in range(BL):
            g, bi = b // 4, b % 4
            # B/C broadcast via mask-matmul + copy: lane 8*n+dl <- row r
            Bbc = cp.tile([128, N], BF16, tag="Bbc", name="Bbc")
            Cbc = cp.tile([128, N], BF16, tag="Cbc", name="Cbc")
            for (mask, dst), tg in (((M_b[bi], Bbc), "pdl"),
                                    ((M_c[bi], Cbc), "pdx")):
                for c0 in range(0, N, 1024):
                    p_b = pmm.tile([128, 1024], F32, tag=tg, name="pbc")
                    for o in (0, 512):
                        nc.tensor.matmul(p_b[:, o:o + 512], mask,
                                         BC[g][:, c0 + o:c0 + o + 512])
                    act(dst[:, c0:c0 + 1024], p_b, AF.Copy)
            hcs = []
            for t in range(3):
                a_t = cp.tile([128, N], BF16, tag="a_t", name="a_t")
                dBx = cp.tile([128, N], BF16, tag="dBx", name="dBx")
                for c0 in range(0, N, 1024):
                    sl = slice(c0, c0 + 1024)
                    p_dl = pmm.tile([128, 1024], F32, tag="pdl", name="p_dl")
                    p_dx = pmm.tile([128, 1024], F32, tag="pdx", name="p_dx")
                    for o in (0, 512):
                        nc.tensor.matmul(p_dl[:, o:o + 512], M_dbc[3 * bi + t],
                                         delta[g][:, c0 + o:c0 + o + 512])
                        nc.tensor.matmul(p_dx[:, o:o + 512], M_dbc[3 * bi + t],
                                         dx[g][:, c0 + o:c0 + o + 512])
                    act(a_t[:, sl], p_dl, AF.Exp, scale=ScA[t])
                    nc.vector.tensor_mul(dBx[:, sl], p_dx, Bbc[:, sl])
                h_t = cp.tile([128, N], BF16, tag="h_t", name="h_t", bufs=1)
                nc.vector.tensor_tensor_scan(h_t, a_t, dBx, 0.0,
                                             OP.mult, OP.add)
                hc = hcp.tile([128, N], BF16, tag=f"hc{t}", name="hc")
                heng = nc.gpsimd
                for c0 in C4:
                    heng.tensor_mul(hc[:, c0:c0 + NC2], h_t[:, c0:c0 + NC2],
                                    Cbc[:, c0:c0 + NC2])
                hcs.append(hc)
            # y = (ys + xi*Dp) * silu(z)
            rr = slice(32 * bi, 32 * bi + 32)
            for c0 in C4:
                sl = slice(c0, c0 + NC2)
                p_yt = pd.tile([32, NC2], F32, tag="pd", name="pyt")
                for t in range(3):
                    nc.tensor.matmul(p_yt, W_mask[t], hcs[t][:, sl],
                                     start=(t == 0), stop=False)
                nc.tensor.matmul(p_yt, W_dp[bi], xi[g][:, sl],
                                 start=False, stop=True)
                nc.vector.tensor_mul(y[g][rr, sl], p_yt, siluz[g][rr, sl])

        _phase_E()
        _phase_DF()

    # Prefer the combined ln+exp ACT table: hide Exp/Ln from all other
    # tables so the table-load pass lands on natural_log_exp_and_others
    # (availability-only metadata; claiming less than reality is safe).
    import concourse.bacc as bacc_mod
    from concourse import mybir as _mb
    _orig_gat = bacc_mod.get_activation_tables

    def _gat(arch):
        t = {k: set(v) for k, v in _orig_gat(arch).items()}
        for name, s in t.items():
            if name != "natural_log_exp_and_others":
                s.discard(_mb.ActivationFunctionType.Exp)
                s.discard(_mb.ActivationFunctionType.Ln)
        return t

    bacc_mod.get_activation_tables = _gat
    try:
        nc.compile()
    finally:
        bacc_mod.get_activation_tables = _orig_gat
    return nc


# ---------------------------------------------------------------- host side
def _host_prep(inputs):
    f32, f16 = np.float32, np.float16
    x = inputs["x"].astype(f32)
    in_proj_w = inputs["in_proj_w"].astype(f32)
    conv_w = inputs["conv_w"].astype(f32)
    conv_b = inputs["conv_b"].astype(f32)
    x_proj_w = inputs["x_proj_w"].astype(f32)
    dt_w = inputs["dt_w"].astype(f32)
    dt_b = inputs["dt_b"].astype(f32)
    A_log = inputs["A_log"].astype(f32)
    Dp = inputs["Dp"].astype(f32)
    out_proj_w = inputs["out_proj_w"].astype(f32)
    ln1_g, ln1_b = inputs["ln1_g"].astype(f32), inputs["ln1_b"].astype(f32)
    ffn_w1, ffn_b1 = inputs["ffn_w1"].astype(f32), inputs["ffn_b1"].astype(f32)
    ffn_w2, ffn_b2 = inputs["ffn_w2"].astype(f32), inputs["ffn_b2"].astype(f32)
    ffn_ln_g = inputs["ffn_ln_g"].astype(f32)
    ffn_ln_b = inputs["ffn_ln_b"].astype(f32)
    cnn_w, cnn_b = inputs["cnn_w"].astype(f32), inputs["cnn_b"].astype(f32)
    lin1_w, lin1_b = inputs["lin1_w"].astype(f32), inputs["lin1_b"].astype(f32)
    lin2_w, lin2_b = inputs["lin2_w"].astype(f32), inputs["lin2_b"].astype(f32)
    lin3_w, lin3_b = inputs["lin3_w"].astype(f32), inputs["lin3_b"].astype(f32)

    sh = {}
    # fused in_proj + conv:  Wxc[k*12+m, d] = conv_w[d,0,k]*in_proj_w[d,m]
    Wxc = np.einsum('dk,dm->kmd', conv_w[:, 0, :], in_proj_w[:DI]).reshape(48, DI)
    sh["w_xc"] = np.zeros((96, 64), f32)
    sh["w_z"] = np.zeros((96, 64), f32)
    for b2 in range(2):
        sh["w_xc"][48 * b2:48 * b2 + 48, 32 * b2:32 * b2 + 24] = Wxc
        for m in range(DM):
            sh["w_z"][48 * b2 + 36 + m, 32 * b2:32 * b2 + 24] = in_proj_w[DI:, m]
    # x_proj (delta rank-1 folded)
    Wdelta = np.einsum('d,j->jd', dt_w[:, 0], x_proj_w[0])     # [24,24]
    WBC = x_proj_w[1:].T                                       # [24,32]
    sh["w_delta"] = np.zeros((128, 128), f32)
    sh["w_bc"] = np.zeros((128, 128), f32)
    for bi in range(4):
        r = slice(32 * bi, 32 * bi + 24)
        sh["w_delta"][r, 32 * bi:32 * bi + 24] = Wdelta
        sh["w_bc"][r, 32 * bi:32 * bi + 32] = WBC
    # out_proj with centering fold
    Pc = np.eye(DM, dtype=f32) - f32(1.0 / DM)
    WopT = (Pc @ out_proj_w).T                                 # [24,12]
    sh["w_op"] = np.zeros((128, 64), f32)
    for bi in range(4):
        sh["w_op"][32 * bi:32 * bi + 24, 16 * bi:16 * bi + 12] = WopT
    sh["w_ones12"] = np.zeros((128, 8), f32)
    sh["w_bc8"] = np.zeros((8, 128), f32)
    for b in range(8):
        sh["w_ones12"][16 * b:16 * b + 12, b] = f32(1.0 / DM)
        sh["w_bc8"][b, 16 * b:16 * b + 16] = 1.0
    # ffn (0.5 of exact-gelu folded into w_ffn2)
    W1p = (ffn_w1 * ln1_g[None, :]).T                          # [12,48]
    b1p = ffn_b1 + ffn_w1 @ ln1_b
    sh["w_ffn1"] = np.zeros((4, 128, 128), f32)
    sh["w_ffn2"] = np.zeros((4, 128, 32), f32)
    for q in range(4):
        for b2 in range(2):
            b = 2 * q + b2
            sh["w_ffn1"][q, 16 * b:16 * b + 12, 64 * b2:64 * b2 + 48] = W1p
            sh["w_ffn2"][q, 64 * b2:64 * b2 + 48,
                         16 * b2:16 * b2 + 12] = 0.5 * ffn_w2.T
    sh["w_pc"] = np.zeros((128, 128), f32)
    W1aT = (lin1_w[:, :DM] * ffn_ln_g[None, :]).T              # [12,12]
    W1bT = lin1_w[:, DM:].T
    sh["w_lin1a"] = np.zeros((128, 128), f32)
    sh["w_lin1b"] = np.zeros((128, 128), f32)
    for b in range(8):
        r = slice(16 * b, 16 * b + 12)
        sh["w_pc"][r, r] = Pc
        sh["w_lin1a"][r, r] = W1aT
        sh["w_lin1b"][r, r] = W1bT
    b1h = lin1_b + lin1_w[:, :DM] @ ffn_ln_b
    b2p = lin2_b - lin2_w.sum(axis=1)
    sh["w_lin2"] = np.zeros((2, 128, 128), f32)
    sh["w_lin3"] = np.zeros((2, 128, 4), f32)
    for g in range(2):
        for bp in range(4):
            b = 4 * g + bp
            sh["w_lin2"][g, 16 * b:16 * b + 12,
                         32 * bp:32 * bp + 20] = lin2_w.T
            sh["w_lin3"][g, 32 * bp:32 * bp + 20, bp] = lin3_w[0]
    sh["w_cnn"] = np.zeros((3, 96, 128), f16)
    for k in range(3):
        for b in range(8):
            sh["w_cnn"][k, 12 * b:12 * b + 12,
                        16 * b:16 * b + 12] = cnn_w[:, :, k].T.astype(f16)
    # scan masks and A scales (lane = 8*n + dl)
    sh["w_mask"] = np.zeros((3, 128, 32), np.float32)
    sh["sc_negA"] = np.zeros((3, 128, 1), f32)
    Asc = -np.exp(A_log)                                       # [24,16]
    for t in range(3):
        for dl in range(8):
            for n in range(DS):
                sh["w_mask"][t, 8 * n + dl, 8 * t + dl] = 1.0
                sh["sc_negA"][t, 8 * n + dl, 0] = Asc[8 * t + dl, n]
    # broadcast masks: 12 (bi,t3) for delta/dx rows + 4 B + 4 C
    masks = np.zeros((128, 20 * 128), f32)
    for bi in range(4):
        for t in range(3):
            i = 3 * bi + t
            for dl in range(8):
                for n in range(DS):
                    masks[32 * bi + 8 * t + dl, 128 * i + 8 * n + dl] = 1.0
        for dl in range(8):
            for n in range(DS):
                masks[32 * bi + n, 128 * (12 + bi) + 8 * n + dl] = 1.0
                masks[32 * bi + 16 + n, 128 * (16 + bi) + 8 * n + dl] = 1.0
    sh["masks"] = masks

    def pack(v, blk, nblk):
        o = np.zeros(128, f32)
        for i in range(nblk):
            o[blk * i:blk * i + len(v)] = v
        return o

    vecs = np.zeros((128, 11), f32)
    bconv64 = np.zeros(64, f32)
    bconv64[0:24] = conv_b
    bconv64[32:56] = conv_b
    vecs[:, 0] = np.concatenate([bconv64, bconv64])
    vecs[:, 1] = pack(dt_b, 32, 4)
    vecs[:, 2] = pack(Dp, 32, 4)
    vecs[:, 3] = pack(ln1_g, 16, 8)
    vecs[:, 4] = pack(ln1_b, 16, 8)
    vecs[:, 5] = pack(b1p, 64, 2)
    vecs[:, 6] = pack(ffn_b2, 16, 8)
    vecs[:, 7] = pack(b1h, 16, 8)
    vecs[:, 8] = pack(b2p, 32, 4)
    vecs[:, 9] = pack(cnn_b, 16, 8)
    vecs[:, 10] = pack(b1p * f32(SQ2I), 64, 2)
    sh["vecs"] = vecs
    sh["w_dp"] = np.zeros((4, 128, 32), f32)
    for bi in range(4):
        for c in range(DI):
            sh["w_dp"][bi, 32 * bi + c, c] = Dp[c]
    sh["b_out"] = np.full((8, 1), lin3_b[0], f32)
    sh["b_eps"] = np.full((8, 1), 1e-12, f32)
    # DFT matrices as moving operand: wdft[cs, kt, t1, f]
    t_ = np.arange(L, dtype=np.float64)
    f_ = np.arange(NF, dtype=np.float64)
    ang = (2 * np.pi / L) * np.outer(t_, f_)                   # [t, f]
    wc = np.cos(ang)
    wsn = np.sin(ang)
    wc[:, 1025:] = 0.0
    wsn[:, 1025:] = 0.0
    wdft = np.zeros((2, NKT, 128, 1024), f16)
    for ch in range(2):
        f0 = 512 * ch
        for kt in range(NKT):
            rows = slice(128 * kt, 128 * kt + 128)
            wdft[ch, kt, :, 0:512] = wc[rows, f0:f0 + 512].astype(f16)
            wdft[ch, kt, :, 512:1024] = wsn[rows, f0:f0 + 512].astype(f16)
    sh["wdft"] = wdft
    nyc = np.zeros((128, NKT), f16)
    for kt in range(NKT):
        nyc[:, kt] = wc[128 * kt:128 * kt + 128, 1024].astype(f16)
    sh["nyc"] = nyc

    import ml_dtypes
    for k in ("w_mask", "w_xc", "w_z", "w_delta", "w_bc", "w_dp", "w_op",
              "w_ffn1", "w_ffn2", "w_pc", "w_lin1a", "w_lin1b", "w_lin2",
              "w_lin3", "masks", "w_ones12"):
        sh[k] = sh[k].astype(ml_dtypes.bfloat16)

    # per-core data
    per_core = []
    for c in range(NCORES):
        xl = x[BL * c:BL * c + BL]                             # [8,2048,12]
        xs = np.zeros((4, 96, N), f32)
        for j in range(4):
            for b2 in range(2):
                xb = xl[2 * j + b2]                            # [2048,12]
                for k in range(4):
                    shf = 3 - k
                    r0 = 48 * b2 + 12 * k
                    if shf == 0:
                        xs[j, r0:r0 + 12, :] = xb.T
                    else:
                        xs[j, r0:r0 + 12, shf:] = xb[:-shf].T
        xt = np.zeros((128, NKT * 96), f16)
        for kt in range(NKT):
            xt[:, 96 * kt:96 * kt + 96] = \
                xl[:, 128 * kt:128 * kt + 128].transpose(1, 0, 2) \
                .reshape(128, 96).astype(f16)
        per_core.append({"xs": xs.astype(ml_dtypes.bfloat16), "xt": xt})
    return sh, per_core


def kernel(**inputs):
    sh, per_core = _host_prep(inputs)
    if "nc" not in _CACHE:
        _CACHE["nc"] = _build_module()
    nc = _CACHE["nc"]
    in_maps = [{**sh, **pc} for pc in per_core]
    from concourse.bass_utils import run_bass_kernel_spmd
    res = run_bass_kernel_spmd(nc, in_maps, core_ids=list(range(NCORES)))
    outs = [res.results[c]["out"].reshape(BL) for c in range(NCORES)]
    return np.concatenate(outs).astype(np.float32)


# revision 24
# speedup vs baseline: 1.2534x; 1.2534x over previous
"""Trainium2 Bass kernel for nn_Net_90331752170289 (Mamba block + FFT/CNN + fusion head).

Strategy: pure data parallelism over batch (8 batches per core on 8 cores).

v4: phase-C broadcasts (delta/dx/B/C fan-outs) are mask-matmuls into PSUM
(SBUF->SBUF broadcast DMAs cost ~500ns per 4KB per-partition descriptor and
dominated v1).  The DFT keeps x stationary on the PE and streams the DFT
matrix as the moving operand so |FFT| lands as [96 seqs, freq] directly --
no transposes; the mirror half is a reversed-stride vector copy.  Phases
D (out_proj/LN/FFN/LN2) and F (fusion head) are split per batch-group g so
g=0's chain overlaps the second half of the scan phase.  dBx is copied
PSUM->SBUF on the scalar engine so the B-multiply runs in the DVE 2x mode.
hc multiplies ride the otherwise-idle GPSIMD.  Most tensors are bf16.

Lane layout for scan tiles: lane = 8*n + dl  (d = 8*t3 + dl).
X24 tensors (xi, siluz, delta, dx, BC): [128, 2048] per g, row = 32*bi + ch.
X12 tensors (hhat, s_t, xm_hat, xcnn): [64, 2048] per g, row = 16*b' + m.
"""
import numpy as np

B, L, DM = 64, 2048, 12
DI, DS, DC = 24, 16, 4
NCORES = 8
BL = B // NCORES          # 8 local batches per core
N = L                     # free dim per batch
NC2 = 512                 # psum chunk (1 bank)
NF = 1152                 # padded rfft bins (valid 0..1024)
NKT = L // 128            # 16 DFT K-tiles
SQ2I = 0.7071067811865476

_CACHE = {}


# ---------------------------------------------------------------- device code
def _build_module():
    import concourse.bacc as bacc
    import concourse.bass as bass
    import concourse.tile as tile
    from concourse import mybir
    from contextlib import ExitStack

    F32 = mybir.dt.float32
    F32R = mybir.dt.float32r
    F16 = mybir.dt.float16
    BF16 = mybir.dt.bfloat16
    AF = mybir.ActivationFunctionType
    OP = mybir.AluOpType
    AX = mybir.AxisListType

    nc = bacc.Bacc("TRN2", target_bir_lowering=False, debug=False)

    def din(name, shape, dt=F32R):
        return nc.dram_tensor(name, shape, dt, kind="ExternalInput")

    # per-core data
    xs_d = din("xs", [4, 96, N], BF16)            # in_proj rhs, per b-pair
    xt_d = din("xt", [128, NKT * 96], F16)        # DFT lhsT, kt-major cols
    wdft_d = din("wdft", [2, NKT, 128, 1024], F16)  # [fchunk, kt, t1, cs*f]
    nyc_d = din("nyc", [128, NKT], F16)           # (-1)^t cos col per kt
    # folded weights (identical on all cores)
    w_xc_d = din("w_xc", [96, 64], BF16)
    w_z_d = din("w_z", [96, 64], BF16)
    w_delta_d = din("w_delta", [128, 128], BF16)
    w_bc_d = din("w_bc", [128, 128], BF16)
    w_op_d = din("w_op", [128, 64], BF16)
    w_ones12_d = din("w_ones12", [128, 8], BF16)
    w_bc8_d = din("w_bc8", [8, 128])
    w_ffn1_d = din("w_ffn1", [4, 128, 128], BF16)
    w_ffn2_d = din("w_ffn2", [4, 128, 32], BF16)
    w_pc_d = din("w_pc", [128, 128], BF16)
    w_lin1a_d = din("w_lin1a", [128, 128], BF16)
    w_lin1b_d = din("w_lin1b", [128, 128], BF16)
    w_lin2_d = din("w_lin2", [2, 128, 128], BF16)
    w_lin3_d = din("w_lin3", [2, 128, 4], BF16)
    w_cnn_d = din("w_cnn", [3, 96, 128], F16)
    w_mask_d = din("w_mask", [3, 128, 32], BF16)
    masks_d = din("masks", [128, 20 * 128], BF16)  # 12 dbc + 4 B + 4 C
    sc_negA_d = din("sc_negA", [3, 128, 1], F32)
    vec_d = din("vecs", [128, 11], F32)           # packed per-partition vectors
    w_dp_d = din("w_dp", [4, 128, 32], BF16)
    b_out_d = din("b_out", [8, 1], F32)
    b_eps_d = din("b_eps", [8, 1], F32)
    (V_BCONV, V_BDT, V_SDP, V_G1, V_B1, V_BFFN1, V_BFFN2, V_BHEAD1,
     V_BLIN2, V_BCNN, V_BFFN1S) = range(11)

    out_d = nc.dram_tensor("out", [8, 1], F32, kind="ExternalOutput")

    with tile.TileContext(nc) as tc, ExitStack() as ctx:
        sg = ctx.enter_context(tc.tile_pool(name="singles", bufs=1))
        ws = ctx.enter_context(tc.tile_pool(name="work", bufs=2))
        big = ctx.enter_context(tc.tile_pool(name="big", bufs=1))
        cp = ctx.enter_context(tc.tile_pool(name="cpool", bufs=2))
        hcp = ctx.enter_context(tc.tile_pool(name="hcpool", bufs=1))
        wdp = ctx.enter_context(tc.tile_pool(name="wdftpool", bufs=3))
        pmm = ctx.enter_context(tc.tile_pool(name="pmm", bufs=1, space="PSUM"))
        pd = ctx.enter_context(tc.tile_pool(name="pd", bufs=2, space="PSUM"))
        pdft = ctx.enter_context(tc.tile_pool(name="pdft", bufs=1,
                                              space="PSUM"))

        def load(dram_ap, shape, dt, tag, pool=sg):
            t = pool.tile(shape, dt, tag=tag, name=tag)
            nc.sync.dma_start(out=t, in_=dram_ap)
            return t

        def act(out, in_, func, **kw):
            return nc.scalar.activation(out, in_, func, **kw)

        # ---- load weights/constants into SBUF
        W_xc = load(w_xc_d[:, :], [96, 64], BF16, "w_xc")
        W_z = load(w_z_d[:, :], [96, 64], BF16, "w_z")
        W_delta = load(w_delta_d[:, :], [128, 128], BF16, "w_delta")
        W_bc = load(w_bc_d[:, :], [128, 128], BF16, "w_bc")
        W_op = load(w_op_d[:, :], [128, 64], BF16, "w_op")
        W_ones12 = load(w_ones12_d[:, :], [128, 8], BF16, "w_ones12")
        W_bc8 = load(w_bc8_d[:, :], [8, 128], F32R, "w_bc8")
        W_ffn1 = [load(w_ffn1_d[q], [128, 128], BF16, f"w_ffn1_{q}")
                  for q in range(4)]
        W_ffn2 = [load(w_ffn2_d[q], [128, 32], BF16, f"w_ffn2_{q}")
                  for q in range(4)]
        W_pc = load(w_pc_d[:, :], [128, 128], BF16, "w_pc")
        W_lin1a = load(w_lin1a_d[:, :], [128, 128], BF16, "w_lin1a")
        W_lin1b = load(w_lin1b_d[:, :], [128, 128], BF16, "w_lin1b")
        W_lin2 = [load(w_lin2_d[g], [128, 128], BF16, f"w_lin2_{g}")
                  for g in range(2)]
        W_lin3 = [load(w_lin3_d[g], [128, 4], BF16, f"w_lin3_{g}")
                  for g in range(2)]
        W_cnn = [load(w_cnn_d[k], [96, 128], F16, f"w_cnn_{k}")
                 for k in range(3)]
        W_mask = [load(w_mask_d[t], [128, 32], BF16, f"w_mask_{t}")
                  for t in range(3)]
        Masks = sg.tile([128, 20 * 128], BF16, tag="masks", name="masks")
        for i in range(4):
            nc.sync.dma_start(out=Masks[:, 640 * i:640 * i + 640],
                              in_=masks_d[:, 640 * i:640 * i + 640])
        M_dbc = [Masks[:, 128 * i:128 * i + 128] for i in range(12)]
        M_b = [Masks[:, 128 * (12 + i):128 * (12 + i) + 128] for i in range(4)]
        M_c = [Masks[:, 128 * (16 + i):128 * (16 + i) + 128] for i in range(4)]
        ScA = [load(sc_negA_d[t], [128, 1], F32, f"scA_{t}") for t in range(3)]
        Vec_t = load(vec_d[:, :], [128, 11], F32, "vec_t")
        Vec = [Vec_t[:, i:i + 1] for i in range(11)]
        W_dp = [load(w_dp_d[bi], [128, 32], BF16, f"w_dp{bi}")
                for bi in range(4)]
        Bout = load(b_out_d[:, :], [8, 1], F32, "b_out")
        Beps = load(b_eps_d[:, :], [8, 1], F32, "b_eps")

        # ---- persistent activations
        xi = [big.tile([128, N], BF16, tag=f"xi{g}", name=f"xi{g}")
              for g in range(2)]
        siluz = [big.tile([128, N], BF16, tag=f"siluz{g}", name=f"siluz{g}")
                 for g in range(2)]
        delta = [big.tile([128, N], BF16, tag=f"delta{g}", name=f"delta{g}")
                 for g in range(2)]
        dx = [big.tile([128, N], BF16, tag=f"dx{g}", name=f"dx{g}")
              for g in range(2)]
        BC = [big.tile([128, N], BF16, tag=f"bc{g}", name=f"bc{g}")
              for g in range(2)]
        y = [big.tile([128, N], BF16, tag=f"y{g}", name=f"y{g}")
             for g in range(2)]
        xfT = big.tile([96, N + 2], F16, tag="xfT", name="xfT")
        racc = [sg.tile([4, 1], F32, tag=f"racc{g}", name=f"racc{g}")
                for g in range(2)]

        C4 = [c * NC2 for c in range(N // NC2)]

        # ================= phase E: DFT |FFT| (x stationary, W moving) =====
        def _phase_E():
            xt_sb = sg.tile([128, NKT * 96], F16, tag="xt", name="xt")
            nc.sync.dma_start(out=xt_sb, in_=xt_d[:, :])
            nyc = load(nyc_d[:, :], [128, NKT], F16, "nyc")
            nc.vector.memset(xfT[:, 0:1], 0.0)
            nc.vector.memset(xfT[:, N + 1:N + 2], 0.0)
            for ch in range(2):
                f0 = 512 * ch
                pc = pdft.tile([96, 512], F32, tag="pdc", name="pdc")
                ps = pdft.tile([96, 512], F32, tag="pds", name="pds")
                for kt in range(NKT):
                    wsl = wdp.tile([128, 1024], F16, tag="wsl", name="wsl")
                    nc.sync.dma_start(out=wsl, in_=wdft_d[ch, kt])
                    xk = xt_sb[:, 96 * kt:96 * kt + 96]
                    nc.tensor.matmul(pc, xk, wsl[:, 0:512],
                                     start=(kt == 0), stop=(kt == NKT - 1))
                    nc.tensor.matmul(ps, xk, wsl[:, 512:1024],
                                     start=(kt == 0), stop=(kt == NKT - 1))
                sqs = ws.tile([96, 512], F32, tag="sqcs", name="sqcs")
                sq2 = ws.tile([96, 512], F32, tag="sqcs2", name="sqcs2")
                act(sqs, pc, AF.Square)
                act(sq2, ps, AF.Square)
                nc.vector.scalar_tensor_tensor(sqs, sqs, 1e-20, sq2,
                                               OP.add, OP.add)
                lnm = ws.tile([96, 512], F32, tag="lnm", name="lnm")
                act(lnm, sqs, AF.Ln)
                act(xfT[:, 1 + f0:1 + f0 + 512], lnm, AF.Exp, scale=0.5)
                # mirror: f in [f0, f0+512) -> cols 1 + (2048 - f), desc
                nsrc = 511 if ch == 0 else 512
                rev = bass.AP(tensor=xfT.tensor,
                              offset=xfT.offset + (2048 if ch == 0 else 1537),
                              ap=[list(xfT.ap[0]), [-1, nsrc]])
                nc.vector.tensor_copy(
                    rev, xfT[:, 2 + f0:2 + f0 + nsrc] if ch == 0
                    else xfT[:, 1 + f0:1 + f0 + nsrc])
            # Nyquist bin f=1024: X = sum_t x[t] cos(pi t); sin part is 0
            pny = pdft.tile([96, 1], F32, tag="pdc", name="pny")
            for kt in range(NKT):
                nc.tensor.matmul(pny, xt_sb[:, 96 * kt:96 * kt + 96],
                                 nyc[:, kt:kt + 1],
                                 start=(kt == 0), stop=(kt == NKT - 1))
            sqn = ws.tile([96, 1], F32, tag="rc", name="sqn")
            act(sqn, pny, AF.Square)
            lnn = ws.tile([96, 1], F32, tag="rc", name="lnn")
            act(lnn, sqn, AF.Ln)
            act(xfT[:, 1025:1026], lnn, AF.Exp, scale=0.5)

        # ================= phases D + F (all 8 batches, rows 16b+m) =========
        def _phase_DF():
            hhat = big.tile([128, N], BF16, tag="hhat", name="hhat")
            h_aff = big.tile([128, N], BF16, tag="h_aff", name="h_aff")
            CH = [(c * 1024, c * 1024 + 1024) for c in range(N // 1024)]

            def mm512(p, lhsT, rhs, c0, c1, start=True, stop=True):
                for o in range(0, c1 - c0, 512):
                    nc.tensor.matmul(p[:, o:o + 512], lhsT,
                                     rhs[:, c0 + o:c0 + o + 512],
                                     start=start, stop=stop)

            for c0, c1 in CH:
                p_m = [pmm.tile([64, 1024], F32, tag=tg, name="p_m")
                       for tg in ("pdl", "pdx")]
                for g in range(2):
                    mm512(p_m[g], W_op, y[g], c0, c1)
                cent = ws.tile([128, 1024], BF16, tag="cent", name="cent")
                sq = ws.tile([128, 1024], BF16, tag="sq", name="sq")
                for g in range(2):
                    gg = slice(64 * g, 64 * g + 64)
                    act(cent[gg, :], p_m[g], AF.Copy)
                    act(sq[gg, :], p_m[g], AF.Square)
                p_v = pmm.tile([8, 1024], F32, tag="pdl", name="p_v")
                mm512(p_v, W_ones12, sq, 0, 1024)
                sd = ws.tile([8, 1024], F32, tag="sd", name="sd")
                act(sd, p_v, AF.Ln, bias=Beps)
                inv = ws.tile([8, 1024], F32R, tag="sd", name="inv")
                act(inv, sd, AF.Exp, scale=-0.5)
                p_b = pmm.tile([128, 1024], F32, tag="pdx", name="p_b")
                mm512(p_b, W_bc8, inv, 0, 1024)
                nc.vector.tensor_mul(hhat[:, c0:c1], cent, p_b)
                nc.vector.tensor_scalar(h_aff[:, c0:c1], hhat[:, c0:c1],
                                        Vec[V_G1], Vec[V_B1],
                                        OP.mult, OP.add)
            # FFN; gelu=0.5*u*(1+erf(u/sqrt2)), 0.5 folded in W_ffn2
            s_t = big.tile([128, N], BF16, tag="s_t", name="s_t")
            for q in range(4):
                for c0, c1 in CH:
                    p_f = pmm.tile([128, 1024], F32, tag="pdl", name="p_f")
                    mm512(p_f, W_ffn1[q], hhat, c0, c1)
                    u_b = ws.tile([128, 1024], BF16, tag="sgm", name="u_b")
                    act(u_b, p_f, AF.Identity, bias=Vec[V_BFFN1])
                    erf_t = ws.tile([128, 1024], BF16, tag="erf", name="erf")
                    act(erf_t, u_b, AF.Erf, scale=SQ2I)
                    ff_c = ws.tile([128, 1024], BF16, tag="ffch", name="ff_c")
                    nc.vector.scalar_tensor_tensor(
                        ff_c, erf_t, 1.0, u_b, OP.add, OP.mult)
                    p_2 = pmm.tile([32, 1024], F32, tag="pdx", name="p_2")
                    mm512(p_2, W_ffn2[q], ff_c, 0, 1024)
                    rq = slice(32 * q, 32 * q + 32)
                    nc.vector.scalar_tensor_tensor(
                        s_t[rq, c0:c1], p_2, Vec[V_BFFN2][rq, :],
                        h_aff[rq, c0:c1], OP.add, OP.add)
            # LN2
            xm_hat = big.tile([128, N], BF16, tag="xm_hat", name="xm_hat")
            for c0, c1 in CH:
                p_c = pmm.tile([128, 1024], F32, tag="pdl", name="p_c")
                mm512(p_c, W_pc, s_t, c0, c1)
                c2 = ws.tile([128, 1024], BF16, tag="cent", name="c2")
                act(c2, p_c, AF.Copy)
                sq2 = ws.tile([128, 1024], BF16, tag="sq", name="sq2")
                act(sq2, p_c, AF.Square)
                p_v2 = pmm.tile([8, 1024], F32, tag="pdx", name="p_v2")
                mm512(p_v2, W_ones12, sq2, 0, 1024)
                sd2 = ws.tile([8, 1024], F32, tag="sd", name="sd2")
                act(sd2, p_v2, AF.Ln, bias=Beps)
                inv2 = ws.tile([8, 1024], F32R, tag="sd", name="inv2")
                act(inv2, sd2, AF.Exp, scale=-0.5)
                p_b2 = pmm.tile([128, 1024], F32, tag="pdl", name="p_b2")
                mm512(p_b2, W_bc8, inv2, 0, 1024)
                nc.vector.tensor_mul(xm_hat[:, c0:c1], c2, p_b2)
            # CNN (xfT ready early; 3 shifted block-diag matmuls)
            xcnn = big.tile([128, N], BF16, tag="xcnn", name="xcnn")
            for c0, c1 in CH:
                p_cn = pmm.tile([128, 1024], F32, tag="pdx", name="p_cn")
                for k in range(3):
                    mm512(p_cn, W_cnn[k], xfT, c0 + k, c1 + k,
                          start=(k == 0), stop=(k == 2))
                act(xcnn[:, c0:c1], p_cn, AF.Identity, bias=Vec[V_BCNN])
            # fusion head
            for g in range(2):
                nc.vector.memset(racc[g], 0.0)
            for c0, c1 in CH:
                p_1 = pmm.tile([128, 1024], F32, tag="pdl", name="p_1")
                mm512(p_1, W_lin1a, xm_hat, c0, c1, start=True, stop=False)
                mm512(p_1, W_lin1b, xcnn, c0, c1, start=False, stop=True)
                mneg = ws.tile([128, 1024], BF16, tag="mneg", name="mneg")
                nc.vector.tensor_scalar(mneg, p_1, Vec[V_BHEAD1], 0.0,
                                        OP.add, OP.min)
                e_t = ws.tile([128, 1024], BF16, tag="e_t", name="e_t")
                act(e_t, mneg, AF.Exp)
                r_t = ws.tile([128, 1024], BF16, tag="r_t", name="r_t")
                act(r_t, p_1, AF.Relu, bias=Vec[V_BHEAD1])
                v_t = ws.tile([128, 1024], BF16, tag="e_t", name="v_t")
                nc.vector.tensor_add(v_t, r_t, e_t)
                for g in range(2):
                    p_o2 = pmm.tile([128, 1024], F32, tag="pdx", name="p_o2")
                    mm512(p_o2, W_lin2[g], v_t, 0, 1024)
                    o2c = ws.tile([128, 1024], BF16, tag="mneg", name="o2c")
                    act(o2c, p_o2, AF.Identity, bias=Vec[V_BLIN2])
                    p_o3 = pmm.tile([4, 1024], F32, tag="pdl", name="p_o3")
                    mm512(p_o3, W_lin3[g], o2c, 0, 1024)
                    o3c = ws.tile([4, 1024], F32, tag="sd", name="o3c")
                    act(o3c, p_o3, AF.Copy)
                    rc = ws.tile([4, 1], F32, tag="rc", name="rc")
                    nc.vector.tensor_reduce(rc, o3c, AX.X, OP.add)
                    nc.vector.tensor_add(racc[g], racc[g], rc)
            for g in range(2):
                res = sg.tile([4, 1], F32, tag=f"res{g}", name=f"res{g}")
                act(res, racc[g], AF.Sigmoid, bias=Bout[0:4, :],
                    scale=1.0 / N)
                nc.sync.dma_start(out=out_d[4 * g:4 * g + 4, :], in_=res)

        # ================= phase A: fused in_proj + causal conv, silu =======
        for g in range(2):
            xsp = [ws.tile([96, N], BF16, tag="pairA", name="xsp")
                   for j in range(2)]
            for j in range(2):
                for ph in range(2):
                    nc.sync.dma_start(out=xsp[j][48 * ph:48 * ph + 48, :],
                                      in_=xs_d[2 * g + j, 48 * ph:48 * ph + 48])
            for c0 in range(0, N, 1024):
                sl = slice(c0, c0 + 1024)
                for j in range(2):
                    jj = slice(64 * j, 64 * j + 64)
                    p_xc = pmm.tile([64, 1024], F32, tag="pdl", name="pmm")
                    for o in (0, 512):
                        nc.tensor.matmul(p_xc[:, o:o + 512], W_xc,
                                         xsp[j][:, c0 + o:c0 + o + 512])
                    act(xi[g][jj, sl], p_xc, AF.Silu,
                        bias=Vec[V_BCONV][jj, :])
                    p_z = pmm.tile([64, 1024], F32, tag="pdx", name="pmm")
                    for o in (0, 512):
                        nc.tensor.matmul(p_z[:, o:o + 512], W_z,
                                         xsp[j][:, c0 + o:c0 + o + 512])
                    act(siluz[g][jj, sl], p_z, AF.Silu)

        # ================= phase B: x_proj (delta folded), dx ==============
        for g in range(2):
            for c0 in range(0, N, 1024):
                sl = slice(c0, c0 + 1024)
                p_d = pmm.tile([128, 1024], F32, tag="pdl", name="pmm")
                p_bc = pmm.tile([128, 1024], F32, tag="pdx", name="pmm")
                for o in (0, 512):
                    nc.tensor.matmul(p_d[:, o:o + 512], W_delta,
                                     xi[g][:, c0 + o:c0 + o + 512])
                    nc.tensor.matmul(p_bc[:, o:o + 512], W_bc,
                                     xi[g][:, c0 + o:c0 + o + 512])
                edt = ws.tile([128, 1024], F32, tag="sgm", name="edt")
                act(edt, p_d, AF.Exp, bias=Vec[V_BDT])
                act(delta[g][:, sl], edt, AF.Ln, bias=1.0)
                act(BC[g][:, sl], p_bc, AF.Copy)
            nc.vector.tensor_mul(dx[g], delta[g], xi[g])

        # ================= phase C: selective scan ==========================
        for b in range(BL):
            g, bi = b // 4, b % 4
            # B/C broadcast via mask-matmul + copy: lane 8*n+dl <- row r
            Bbc = cp.tile([128, N], BF16, tag="Bbc", name="Bbc")
            Cbc = cp.tile([128, N], BF16, tag="Cbc", name="Cbc")
            for (mask, dst), tg in (((M_b[bi], Bbc), "pdl"),
                                    ((M_c[bi], Cbc), "pdx")):
                for c0 in range(0, N, 1024):
                    p_b = pmm.tile([128, 1024], F32, tag=tg, name="pbc")
                    for o in (0, 512):
                        nc.tensor.matmul(p_b[:, o:o + 512], mask,
                                         BC[g][:, c0 + o:c0 + o + 512])
                    act(dst[:, c0:c0 + 1024], p_b, AF.Copy)
            hcs = []
            for t in range(3):
                a_t = cp.tile([128, N], BF16, tag="a_t", name="a_t")
                dBx = cp.tile([128, N], BF16, tag="dBx", name="dBx")
                for c0 in range(0, N, 1024):
                    sl = slice(c0, c0 + 1024)
                    p_dl = pmm.tile([128, 1024], F32, tag="pdl", name="p_dl")
                    p_dx = pmm.tile([128, 1024], F32, tag="pdx", name="p_dx")
                    for o in (0, 512):
                        nc.tensor.matmul(p_dl[:, o:o + 512], M_dbc[3 * bi + t],
                                         delta[g][:, c0 + o:c0 + o + 512])
                        nc.tensor.matmul(p_dx[:, o:o + 512], M_dbc[3 * bi + t],
                                         dx[g][:, c0 + o:c0 + o + 512])
                    act(a_t[:, sl], p_dl, AF.Exp, scale=ScA[t])
                    nc.vector.tensor_mul(dBx[:, sl], p_dx, Bbc[:, sl])
                h_t = cp.tile([128, N], BF16, tag="h_t", name="h_t", bufs=1)
                nc.vector.tensor_tensor_scan(h_t, a_t, dBx, 0.0,
                                             OP.mult, OP.add)
                hc = hcp.tile([128, N], BF16, tag=f"hc{t}", name="hc")
                heng = nc.gpsimd if t < 2 else nc.vector
                for c0 in C4:
                    heng.tensor_mul(hc[:, c0:c0 + NC2], h_t[:, c0:c0 + NC2],
                                    Cbc[:, c0:c0 + NC2])
                hcs.append(hc)
            # y = (ys + xi*Dp) * silu(z)
            rr = slice(32 * bi, 32 * bi + 32)
            for c0 in C4:
                sl = slice(c0, c0 + NC2)
                p_yt = pd.tile([32, NC2], F32, tag="pd", name="pyt")
                for t in range(3):
                    nc.tensor.matmul(p_yt, W_mask[t], hcs[t][:, sl],
                                     start=(t == 0), stop=False)
                nc.tensor.matmul(p_yt, W_dp[bi], xi[g][:, sl],
                                 start=False, stop=True)
                nc.vector.tensor_mul(y[g][rr, sl], p_yt, siluz[g][rr, sl])

        _phase_E()
        _phase_DF()

    # Prefer the combined ln+exp ACT table: hide Exp/Ln from all other
    # tables so the table-load pass lands on natural_log_exp_and_others
    # (availability-only metadata; claiming less than reality is safe).
    import concourse.bacc as bacc_mod
    from concourse import mybir as _mb
    _orig_gat = bacc_mod.get_activation_tables

    def _gat(arch):
        t = {k: set(v) for k, v in _orig_gat(arch).items()}
        for name, s in t.items():
            if name != "natural_log_exp_and_others":
                s.discard(_mb.ActivationFunctionType.Exp)
                s.discard(_mb.ActivationFunctionType.Ln)
        return t

    bacc_mod.get_activation_tables = _gat
    try:
        nc.compile()
    finally:
        bacc_mod.get_activation_tables = _orig_gat
    return nc


# ---------------------------------------------------------------- host side
def _host_prep(inputs):
    f32, f16 = np.float32, np.float16
    x = inputs["x"].astype(f32)
    in_proj_w = inputs["in_proj_w"].astype(f32)
    conv_w = inputs["conv_w"].astype(f32)
    conv_b = inputs["conv_b"].astype(f32)
    x_proj_w = inputs["x_proj_w"].astype(f32)
    dt_w = inputs["dt_w"].astype(f32)
    dt_b = inputs["dt_b"].astype(f32)
    A_log = inputs["A_log"].astype(f32)
    Dp = inputs["Dp"].astype(f32)
    out_proj_w = inputs["out_proj_w"].astype(f32)
    ln1_g, ln1_b = inputs["ln1_g"].astype(f32), inputs["ln1_b"].astype(f32)
    ffn_w1, ffn_b1 = inputs["ffn_w1"].astype(f32), inputs["ffn_b1"].astype(f32)
    ffn_w2, ffn_b2 = inputs["ffn_w2"].astype(f32), inputs["ffn_b2"].astype(f32)
    ffn_ln_g = inputs["ffn_ln_g"].astype(f32)
    ffn_ln_b = inputs["ffn_ln_b"].astype(f32)
    cnn_w, cnn_b = inputs["cnn_w"].astype(f32), inputs["cnn_b"].astype(f32)
    lin1_w, lin1_b = inputs["lin1_w"].astype(f32), inputs["lin1_b"].astype(f32)
    lin2_w, lin2_b = inputs["lin2_w"].astype(f32), inputs["lin2_b"].astype(f32)
    lin3_w, lin3_b = inputs["lin3_w"].astype(f32), inputs["lin3_b"].astype(f32)

    sh = {}
    # fused in_proj + conv:  Wxc[k*12+m, d] = conv_w[d,0,k]*in_proj_w[d,m]
    Wxc = np.einsum('dk,dm->kmd', conv_w[:, 0, :], in_proj_w[:DI]).reshape(48, DI)
    sh["w_xc"] = np.zeros((96, 64), f32)
    sh["w_z"] = np.zeros((96, 64), f32)
    for b2 in range(2):
        sh["w_xc"][48 * b2:48 * b2 + 48, 32 * b2:32 * b2 + 24] = Wxc
        for m in range(DM):
            sh["w_z"][48 * b2 + 36 + m, 32 * b2:32 * b2 + 24] = in_proj_w[DI:, m]
    # x_proj (delta rank-1 folded)
    Wdelta = np.einsum('d,j->jd', dt_w[:, 0], x_proj_w[0])     # [24,24]
    WBC = x_proj_w[1:].T                                       # [24,32]
    sh["w_delta"] = np.zeros((128, 128), f32)
    sh["w_bc"] = np.zeros((128, 128), f32)
    for bi in range(4):
        r = slice(32 * bi, 32 * bi + 24)
        sh["w_delta"][r, 32 * bi:32 * bi + 24] = Wdelta
        sh["w_bc"][r, 32 * bi:32 * bi + 32] = WBC
    # out_proj with centering fold
    Pc = np.eye(DM, dtype=f32) - f32(1.0 / DM)
    WopT = (Pc @ out_proj_w).T                                 # [24,12]
    sh["w_op"] = np.zeros((128, 64), f32)
    for bi in range(4):
        sh["w_op"][32 * bi:32 * bi + 24, 16 * bi:16 * bi + 12] = WopT
    sh["w_ones12"] = np.zeros((128, 8), f32)
    sh["w_bc8"] = np.zeros((8, 128), f32)
    for b in range(8):
        sh["w_ones12"][16 * b:16 * b + 12, b] = f32(1.0 / DM)
        sh["w_bc8"][b, 16 * b:16 * b + 16] = 1.0
    # ffn (0.5 of exact-gelu folded into w_ffn2)
    W1p = (ffn_w1 * ln1_g[None, :]).T                          # [12,48]
    b1p = ffn_b1 + ffn_w1 @ ln1_b
    sh["w_ffn1"] = np.zeros((4, 128, 128), f32)
    sh["w_ffn2"] = np.zeros((4, 128, 32), f32)
    for q in range(4):
        for b2 in range(2):
            b = 2 * q + b2
            sh["w_ffn1"][q, 16 * b:16 * b + 12, 64 * b2:64 * b2 + 48] = W1p
            sh["w_ffn2"][q, 64 * b2:64 * b2 + 48,
                         16 * b2:16 * b2 + 12] = 0.5 * ffn_w2.T
    sh["w_pc"] = np.zeros((128, 128), f32)
    W1aT = (lin1_w[:, :DM] * ffn_ln_g[None, :]).T              # [12,12]
    W1bT = lin1_w[:, DM:].T
    sh["w_lin1a"] = np.zeros((128, 128), f32)
    sh["w_lin1b"] = np.zeros((128, 128), f32)
    for b in range(8):
        r = slice(16 * b, 16 * b + 12)
        sh["w_pc"][r, r] = Pc
        sh["w_lin1a"][r, r] = W1aT
        sh["w_lin1b"][r, r] = W1bT
    b1h = lin1_b + lin1_w[:, :DM] @ ffn_ln_b
    b2p = lin2_b - lin2_w.sum(axis=1)
    sh["w_lin2"] = np.zeros((2, 128, 128), f32)
    sh["w_lin3"] = np.zeros((2, 128, 4), f32)
    for g in range(2):
        for bp in range(4):
            b = 4 * g + bp
            sh["w_lin2"][g, 16 * b:16 * b + 12,
                         32 * bp:32 * bp + 20] = lin2_w.T
            sh["w_lin3"][g, 32 * bp:32 * bp + 20, bp] = lin3_w[0]
    sh["w_cnn"] = np.zeros((3, 96, 128), f16)
    for k in range(3):
        for b in range(8):
            sh["w_cnn"][k, 12 * b:12 * b + 12,
                        16 * b:16 * b + 12] = cnn_w[:, :, k].T.astype(f16)
    # scan masks and A scales (lane = 8*n + dl)
    sh["w_mask"] = np.zeros((3, 128, 32), np.float32)
    sh["sc_negA"] = np.zeros((3, 128, 1), f32)
    Asc = -np.exp(A_log)                                       # [24,16]
    for t in range(3):
        for dl in range(8):
            for n in range(DS):
                sh["w_mask"][t, 8 * n + dl, 8 * t + dl] = 1.0
                sh["sc_negA"][t, 8 * n + dl, 0] = Asc[8 * t + dl, n]
    # broadcast masks: 12 (bi,t3) for delta/dx rows + 4 B + 4 C
    masks = np.zeros((128, 20 * 128), f32)
    for bi in range(4):
        for t in range(3):
            i = 3 * bi + t
            for dl in range(8):
                for n in range(DS):
                    masks[32 * bi + 8 * t + dl, 128 * i + 8 * n + dl] = 1.0
        for dl in range(8):
            for n in range(DS):
                masks[32 * bi + n, 128 * (12 + bi) + 8 * n + dl] = 1.0
                masks[32 * bi + 16 + n, 128 * (16 + bi) + 8 * n + dl] = 1.0
    sh["masks"] = masks

    def pack(v, blk, nblk):
        o = np.zeros(128, f32)
        for i in range(nblk):
            o[blk * i:blk * i + len(v)] = v
        return o

    vecs = np.zeros((128, 11), f32)
    bconv64 = np.zeros(64, f32)
    bconv64[0:24] = conv_b
    bconv64[32:56] = conv_b
    vecs[:, 0] = np.concatenate([bconv64, bconv64])
    vecs[:, 1] = pack(dt_b, 32, 4)
    vecs[:, 2] = pack(Dp, 32, 4)
    vecs[:, 3] = pack(ln1_g, 16, 8)
    vecs[:, 4] = pack(ln1_b, 16, 8)
    vecs[:, 5] = pack(b1p, 64, 2)
    vecs[:, 6] = pack(ffn_b2, 16, 8)
    vecs[:, 7] = pack(b1h, 16, 8)
    vecs[:, 8] = pack(b2p, 32, 4)
    vecs[:, 9] = pack(cnn_b, 16, 8)
    vecs[:, 10] = pack(b1p * f32(SQ2I), 64, 2)
    sh["vecs"] = vecs
    sh["w_dp"] = np.zeros((4, 128, 32), f32)
    for bi in range(4):
        for c in range(DI):
            sh["w_dp"][bi, 32 * bi + c, c] = Dp[c]
    sh["b_out"] = np.full((8, 1), lin3_b[0], f32)
    sh["b_eps"] = np.full((8, 1), 1e-12, f32)
    # DFT matrices as moving operand: wdft[cs, kt, t1, f]
    t_ = np.arange(L, dtype=np.float64)
    f_ = np.arange(NF, dtype=np.float64)
    ang = (2 * np.pi / L) * np.outer(t_, f_)                   # [t, f]
    wc = np.cos(ang)
    wsn = np.sin(ang)
    wc[:, 1025:] = 0.0
    wsn[:, 1025:] = 0.0
    wdft = np.zeros((2, NKT, 128, 1024), f16)
    for ch in range(2):
        f0 = 512 * ch
        for kt in range(NKT):
            rows = slice(128 * kt, 128 * kt + 128)
            wdft[ch, kt, :, 0:512] = wc[rows, f0:f0 + 512].astype(f16)
            wdft[ch, kt, :, 512:1024] = wsn[rows, f0:f0 + 512].astype(f16)
    sh["wdft"] = wdft
    nyc = np.zeros((128, NKT), f16)
    for kt in range(NKT):
        nyc[:, kt] = wc[128 * kt:128 * kt + 128, 1024].astype(f16)
    sh["nyc"] = nyc

    import ml_dtypes
    for k in ("w_mask", "w_xc", "w_z", "w_delta", "w_bc", "w_dp", "w_op",
              "w_ffn1", "w_ffn2", "w_pc", "w_lin1a", "w_lin1b", "w_lin2",
              "w_lin3", "masks", "w_ones12"):
        sh[k] = sh[k].astype(ml_dtypes.bfloat16)

    # per-core data
    per_core = []
    for c in range(NCORES):
        xl = x[BL * c:BL * c + BL]                             # [8,2048,12]
        xs = np.zeros((4, 96, N), f32)
        for j in range(4):
            for b2 in range(2):
                xb = xl[2 * j + b2]                            # [2048,12]
                for k in range(4):
                    shf = 3 - k
                    r0 = 48 * b2 + 12 * k
                    if shf == 0:
                        xs[j, r0:r0 + 12, :] = xb.T
                    else:
                        xs[j, r0:r0 + 12, shf:] = xb[:-shf].T
        xt = np.zeros((128, NKT * 96), f16)
        for kt in range(NKT):
            xt[:, 96 * kt:96 * kt + 96] = \
                xl[:, 128 * kt:128 * kt + 128].transpose(1, 0, 2) \
                .reshape(128, 96).astype(f16)
        per_core.append({"xs": xs.astype(ml_dtypes.bfloat16), "xt": xt})
    return sh, per_core


def kernel(**inputs):
    sh, per_core = _host_prep(inputs)
    if "nc" not in _CACHE:
        _CACHE["nc"] = _build_module()
    nc = _CACHE["nc"]
    in_maps = [{**sh, **pc} for pc in per_core]
    from concourse.bass_utils import run_bass_kernel_spmd
    res = run_bass_kernel_spmd(nc, in_maps, core_ids=list(range(NCORES)))
    outs = [res.results[c]["out"].reshape(BL) for c in range(NCORES)]
    return np.concatenate(outs).astype(np.float32)


# revision 25
# speedup vs baseline: 1.3323x; 1.0629x over previous
"""Trainium2 Bass kernel for nn_Net_90331752170289 (Mamba block + FFT/CNN + fusion head).

Strategy: pure data parallelism over batch (8 batches per core on 8 cores).

v4: phase-C broadcasts (delta/dx/B/C fan-outs) are mask-matmuls into PSUM
(SBUF->SBUF broadcast DMAs cost ~500ns per 4KB per-partition descriptor and
dominated v1).  The DFT keeps x stationary on the PE and streams the DFT
matrix as the moving operand so |FFT| lands as [96 seqs, freq] directly --
no transposes; the mirror half is a reversed-stride vector copy.  Phases
D (out_proj/LN/FFN/LN2) and F (fusion head) are split per batch-group g so
g=0's chain overlaps the second half of the scan phase.  dBx is copied
PSUM->SBUF on the scalar engine so the B-multiply runs in the DVE 2x mode.
hc multiplies ride the otherwise-idle GPSIMD.  Most tensors are bf16.

Lane layout for scan tiles: lane = 8*n + dl  (d = 8*t3 + dl).
X24 tensors (xi, siluz, delta, dx, BC): [128, 2048] per g, row = 32*bi + ch.
X12 tensors (hhat, s_t, xm_hat, xcnn): [64, 2048] per g, row = 16*b' + m.
"""
import numpy as np

B, L, DM = 64, 2048, 12
DI, DS, DC = 24, 16, 4
NCORES = 8
BL = B // NCORES          # 8 local batches per core
N = L                     # free dim per batch
NC2 = 512                 # psum chunk (1 bank)
NF = 1152                 # padded rfft bins (valid 0..1024)
NKT = L // 128            # 16 DFT K-tiles
SQ2I = 0.7071067811865476

_CACHE = {}


# ---------------------------------------------------------------- device code
def _build_module():
    import concourse.bacc as bacc
    import concourse.bass as bass
    import concourse.tile as tile
    from concourse import mybir
    from contextlib import ExitStack

    F32 = mybir.dt.float32
    F32R = mybir.dt.float32r
    F16 = mybir.dt.float16
    BF16 = mybir.dt.bfloat16
    AF = mybir.ActivationFunctionType
    OP = mybir.AluOpType
    AX = mybir.AxisListType

    nc = bacc.Bacc("TRN2", target_bir_lowering=False, debug=False)

    def din(name, shape, dt=F32R):
        return nc.dram_tensor(name, shape, dt, kind="ExternalInput")

    # per-core data
    xs_d = din("xs", [4, 96, N], BF16)            # in_proj rhs, per b-pair
    xt_d = din("xt", [128, NKT * 96], F16)        # DFT lhsT, kt-major cols
    wdft_d = din("wdft", [2, NKT, 128, 1024], F16)  # [fchunk, kt, t1, cs*f]
    nyc_d = din("nyc", [128, NKT], F16)           # (-1)^t cos col per kt
    # folded weights (identical on all cores)
    w_xc_d = din("w_xc", [96, 64], BF16)
    w_z_d = din("w_z", [96, 64], BF16)
    w_delta_d = din("w_delta", [128, 128], BF16)
    w_bc_d = din("w_bc", [128, 128], BF16)
    w_op_d = din("w_op", [128, 64], BF16)
    w_ones12_d = din("w_ones12", [128, 8], BF16)
    w_bc8_d = din("w_bc8", [8, 128])
    w_ffn1_d = din("w_ffn1", [4, 128, 128], BF16)
    w_ffn2_d = din("w_ffn2", [4, 128, 32], BF16)
    w_pc_d = din("w_pc", [128, 128], BF16)
    w_lin1a_d = din("w_lin1a", [128, 128], BF16)
    w_lin1b_d = din("w_lin1b", [128, 128], BF16)
    w_lin2_d = din("w_lin2", [2, 128, 128], BF16)
    w_lin3_d = din("w_lin3", [2, 128, 4], BF16)
    w_cnn_d = din("w_cnn", [3, 96, 128], F16)
    w_mask_d = din("w_mask", [3, 128, 32], BF16)
    masks_d = din("masks", [128, 20 * 128], BF16)  # 12 dbc + 4 B + 4 C
    sc_negA_d = din("sc_negA", [3, 128, 1], F32)
    vec_d = din("vecs", [128, 11], F32)           # packed per-partition vectors
    w_dp_d = din("w_dp", [4, 128, 32], BF16)
    b_out_d = din("b_out", [8, 1], F32)
    b_eps_d = din("b_eps", [8, 1], F32)
    (V_BCONV, V_BDT, V_SDP, V_G1, V_B1, V_BFFN1, V_BFFN2, V_BHEAD1,
     V_BLIN2, V_BCNN, V_BFFN1S) = range(11)

    out_d = nc.dram_tensor("out", [8, 1], F32, kind="ExternalOutput")

    with tile.TileContext(nc) as tc, ExitStack() as ctx:
        sg = ctx.enter_context(tc.tile_pool(name="singles", bufs=1))
        ws = ctx.enter_context(tc.tile_pool(name="work", bufs=2))
        big = ctx.enter_context(tc.tile_pool(name="big", bufs=1))
        cp = ctx.enter_context(tc.tile_pool(name="cpool", bufs=2))
        hcp = ctx.enter_context(tc.tile_pool(name="hcpool", bufs=1))
        wdp = ctx.enter_context(tc.tile_pool(name="wdftpool", bufs=3))
        pmm = ctx.enter_context(tc.tile_pool(name="pmm", bufs=1, space="PSUM"))
        pd = ctx.enter_context(tc.tile_pool(name="pd", bufs=2, space="PSUM"))
        pdft = ctx.enter_context(tc.tile_pool(name="pdft", bufs=1,
                                              space="PSUM"))

        def load(dram_ap, shape, dt, tag, pool=sg):
            t = pool.tile(shape, dt, tag=tag, name=tag)
            nc.sync.dma_start(out=t, in_=dram_ap)
            return t

        def act(out, in_, func, **kw):
            return nc.scalar.activation(out, in_, func, **kw)

        # ---- load weights/constants into SBUF
        W_xc = load(w_xc_d[:, :], [96, 64], BF16, "w_xc")
        W_z = load(w_z_d[:, :], [96, 64], BF16, "w_z")
        W_delta = load(w_delta_d[:, :], [128, 128], BF16, "w_delta")
        W_bc = load(w_bc_d[:, :], [128, 128], BF16, "w_bc")
        W_op = load(w_op_d[:, :], [128, 64], BF16, "w_op")
        W_ones12 = load(w_ones12_d[:, :], [128, 8], BF16, "w_ones12")
        W_bc8 = load(w_bc8_d[:, :], [8, 128], F32R, "w_bc8")
        W_ffn1 = [load(w_ffn1_d[q], [128, 128], BF16, f"w_ffn1_{q}")
                  for q in range(4)]
        W_ffn2 = [load(w_ffn2_d[q], [128, 32], BF16, f"w_ffn2_{q}")
                  for q in range(4)]
        W_pc = load(w_pc_d[:, :], [128, 128], BF16, "w_pc")
        W_lin1a = load(w_lin1a_d[:, :], [128, 128], BF16, "w_lin1a")
        W_lin1b = load(w_lin1b_d[:, :], [128, 128], BF16, "w_lin1b")
        W_lin2 = [load(w_lin2_d[g], [128, 128], BF16, f"w_lin2_{g}")
                  for g in range(2)]
        W_lin3 = [load(w_lin3_d[g], [128, 4], BF16, f"w_lin3_{g}")
                  for g in range(2)]
        W_cnn = [load(w_cnn_d[k], [96, 128], F16, f"w_cnn_{k}")
                 for k in range(3)]
        W_mask = [load(w_mask_d[t], [128, 32], BF16, f"w_mask_{t}")
                  for t in range(3)]
        Masks = sg.tile([128, 20 * 128], BF16, tag="masks", name="masks")
        for i in range(4):
            nc.sync.dma_start(out=Masks[:, 640 * i:640 * i + 640],
                              in_=masks_d[:, 640 * i:640 * i + 640])
        M_dbc = [Masks[:, 128 * i:128 * i + 128] for i in range(12)]
        M_b = [Masks[:, 128 * (12 + i):128 * (12 + i) + 128] for i in range(4)]
        M_c = [Masks[:, 128 * (16 + i):128 * (16 + i) + 128] for i in range(4)]
        ScA = [load(sc_negA_d[t], [128, 1], F32, f"scA_{t}") for t in range(3)]
        Vec_t = load(vec_d[:, :], [128, 11], F32, "vec_t")
        Vec = [Vec_t[:, i:i + 1] for i in range(11)]
        W_dp = [load(w_dp_d[bi], [128, 32], BF16, f"w_dp{bi}")
                for bi in range(4)]
        Bout = load(b_out_d[:, :], [8, 1], F32, "b_out")
        Beps = load(b_eps_d[:, :], [8, 1], F32, "b_eps")

        # ---- persistent activations
        xi = [big.tile([128, N], BF16, tag=f"xi{g}", name=f"xi{g}")
              for g in range(2)]
        siluz = [big.tile([128, N], BF16, tag=f"siluz{g}", name=f"siluz{g}")
                 for g in range(2)]
        delta = [big.tile([128, N], BF16, tag=f"delta{g}", name=f"delta{g}")
                 for g in range(2)]
        dx = [big.tile([128, N], BF16, tag=f"dx{g}", name=f"dx{g}")
              for g in range(2)]
        BC = [big.tile([128, N], BF16, tag=f"bc{g}", name=f"bc{g}")
              for g in range(2)]
        y = [big.tile([128, N], BF16, tag=f"y{g}", name=f"y{g}")
             for g in range(2)]
        xfT = big.tile([96, N + 2], F16, tag="xfT", name="xfT")
        racc = [sg.tile([4, 1], F32, tag=f"racc{g}", name=f"racc{g}")
                for g in range(2)]

        C4 = [c * NC2 for c in range(N // NC2)]

        # ================= phase E: DFT |FFT| (x stationary, W moving) =====
        def _phase_E():
            xt_sb = sg.tile([128, NKT * 96], F16, tag="xt", name="xt")
            nc.sync.dma_start(out=xt_sb, in_=xt_d[:, :])
            nyc = load(nyc_d[:, :], [128, NKT], F16, "nyc")
            nc.vector.memset(xfT[:, 0:1], 0.0)
            nc.vector.memset(xfT[:, N + 1:N + 2], 0.0)
            for ch in range(2):
                f0 = 512 * ch
                pc = pdft.tile([96, 512], F32, tag="pdc", name="pdc")
                ps = pdft.tile([96, 512], F32, tag="pds", name="pds")
                for kt in range(NKT):
                    wsl = wdp.tile([128, 1024], F16, tag="wsl", name="wsl")
                    nc.sync.dma_start(out=wsl, in_=wdft_d[ch, kt])
                    xk = xt_sb[:, 96 * kt:96 * kt + 96]
                    nc.tensor.matmul(pc, xk, wsl[:, 0:512],
                                     start=(kt == 0), stop=(kt == NKT - 1))
                    nc.tensor.matmul(ps, xk, wsl[:, 512:1024],
                                     start=(kt == 0), stop=(kt == NKT - 1))
                sqs = ws.tile([96, 512], F32, tag="sqcs", name="sqcs")
                sq2 = ws.tile([96, 512], F32, tag="sqcs2", name="sqcs2")
                act(sqs, pc, AF.Square)
                act(sq2, ps, AF.Square)
                nc.vector.scalar_tensor_tensor(sqs, sqs, 1e-20, sq2,
                                               OP.add, OP.add)
                lnm = ws.tile([96, 512], F32, tag="lnm", name="lnm")
                act(lnm, sqs, AF.Ln)
                act(xfT[:, 1 + f0:1 + f0 + 512], lnm, AF.Exp, scale=0.5)
                # mirror: f in [f0, f0+512) -> cols 1 + (2048 - f), desc
                nsrc = 511 if ch == 0 else 512
                rev = bass.AP(tensor=xfT.tensor,
                              offset=xfT.offset + (2048 if ch == 0 else 1537),
                              ap=[list(xfT.ap[0]), [-1, nsrc]])
                nc.vector.tensor_copy(
                    rev, xfT[:, 2 + f0:2 + f0 + nsrc] if ch == 0
                    else xfT[:, 1 + f0:1 + f0 + nsrc])
            # Nyquist bin f=1024: X = sum_t x[t] cos(pi t); sin part is 0
            pny = pdft.tile([96, 1], F32, tag="pdc", name="pny")
            for kt in range(NKT):
                nc.tensor.matmul(pny, xt_sb[:, 96 * kt:96 * kt + 96],
                                 nyc[:, kt:kt + 1],
                                 start=(kt == 0), stop=(kt == NKT - 1))
            sqn = ws.tile([96, 1], F32, tag="rc", name="sqn")
            act(sqn, pny, AF.Square)
            lnn = ws.tile([96, 1], F32, tag="rc", name="lnn")
            act(lnn, sqn, AF.Ln)
            act(xfT[:, 1025:1026], lnn, AF.Exp, scale=0.5)

        # ================= phases D + F (all 8 batches, rows 16b+m) =========
        def _phase_DF():
            hhat = big.tile([128, N], BF16, tag="hhat", name="hhat")
            h_aff = big.tile([128, N], BF16, tag="h_aff", name="h_aff")
            CH = [(c * 1024, c * 1024 + 1024) for c in range(N // 1024)]

            def mm512(p, lhsT, rhs, c0, c1, start=True, stop=True):
                for o in range(0, c1 - c0, 512):
                    nc.tensor.matmul(p[:, o:o + 512], lhsT,
                                     rhs[:, c0 + o:c0 + o + 512],
                                     start=start, stop=stop)

            for c0, c1 in CH:
                p_m = [pmm.tile([64, 1024], F32, tag=tg, name="p_m")
                       for tg in ("pdl", "pdx")]
                for g in range(2):
                    mm512(p_m[g], W_op, y[g], c0, c1)
                cent = ws.tile([128, 1024], BF16, tag="cent", name="cent")
                sq = ws.tile([128, 1024], BF16, tag="sq", name="sq")
                for g in range(2):
                    gg = slice(64 * g, 64 * g + 64)
                    act(cent[gg, :], p_m[g], AF.Copy)
                    act(sq[gg, :], p_m[g], AF.Square)
                p_v = pmm.tile([8, 1024], F32, tag="pdl", name="p_v")
                mm512(p_v, W_ones12, sq, 0, 1024)
                sd = ws.tile([8, 1024], F32, tag="sd", name="sd")
                act(sd, p_v, AF.Ln, bias=Beps)
                inv = ws.tile([8, 1024], F32R, tag="sd", name="inv")
                act(inv, sd, AF.Exp, scale=-0.5)
                p_b = pmm.tile([128, 1024], F32, tag="pdx", name="p_b")
                mm512(p_b, W_bc8, inv, 0, 1024)
                nc.vector.tensor_mul(hhat[:, c0:c1], cent, p_b)
                nc.vector.tensor_scalar(h_aff[:, c0:c1], hhat[:, c0:c1],
                                        Vec[V_G1], Vec[V_B1],
                                        OP.mult, OP.add)
            # FFN; gelu=0.5*u*(1+erf(u/sqrt2)), 0.5 folded in W_ffn2
            s_t = big.tile([128, N], BF16, tag="s_t", name="s_t")
            for q in range(4):
                for c0, c1 in CH:
                    p_f = pmm.tile([128, 1024], F32, tag="pdl", name="p_f")
                    mm512(p_f, W_ffn1[q], hhat, c0, c1)
                    u_b = ws.tile([128, 1024], BF16, tag="sgm", name="u_b")
                    act(u_b, p_f, AF.Identity, bias=Vec[V_BFFN1])
                    erf_t = ws.tile([128, 1024], BF16, tag="erf", name="erf")
                    act(erf_t, u_b, AF.Erf, scale=SQ2I)
                    ff_c = ws.tile([128, 1024], BF16, tag="ffch", name="ff_c")
                    nc.vector.scalar_tensor_tensor(
                        ff_c, erf_t, 1.0, u_b, OP.add, OP.mult)
                    p_2 = pmm.tile([32, 1024], F32, tag="pdx", name="p_2")
                    mm512(p_2, W_ffn2[q], ff_c, 0, 1024)
                    rq = slice(32 * q, 32 * q + 32)
                    nc.vector.scalar_tensor_tensor(
                        s_t[rq, c0:c1], p_2, Vec[V_BFFN2][rq, :],
                        h_aff[rq, c0:c1], OP.add, OP.add)
            # LN2
            xm_hat = big.tile([128, N], BF16, tag="xm_hat", name="xm_hat")
            for c0, c1 in CH:
                p_c = pmm.tile([128, 1024], F32, tag="pdl", name="p_c")
                mm512(p_c, W_pc, s_t, c0, c1)
                c2 = ws.tile([128, 1024], BF16, tag="cent", name="c2")
                act(c2, p_c, AF.Copy)
                sq2 = ws.tile([128, 1024], BF16, tag="sq", name="sq2")
                act(sq2, p_c, AF.Square)
                p_v2 = pmm.tile([8, 1024], F32, tag="pdx", name="p_v2")
                mm512(p_v2, W_ones12, sq2, 0, 1024)
                sd2 = ws.tile([8, 1024], F32, tag="sd", name="sd2")
                act(sd2, p_v2, AF.Ln, bias=Beps)
                inv2 = ws.tile([8, 1024], F32R, tag="sd", name="inv2")
                act(inv2, sd2, AF.Exp, scale=-0.5)
                p_b2 = pmm.tile([128, 1024], F32, tag="pdl", name="p_b2")
                mm512(p_b2, W_bc8, inv2, 0, 1024)
                nc.vector.tensor_mul(xm_hat[:, c0:c1], c2, p_b2)
            # CNN (xfT ready early; 3 shifted block-diag matmuls)
            xcnn = big.tile([128, N], BF16, tag="xcnn", name="xcnn")
            for c0, c1 in CH:
                p_cn = pmm.tile([128, 1024], F32, tag="pdx", name="p_cn")
                for k in range(3):
                    mm512(p_cn, W_cnn[k], xfT, c0 + k, c1 + k,
                          start=(k == 0), stop=(k == 2))
                act(xcnn[:, c0:c1], p_cn, AF.Identity, bias=Vec[V_BCNN])
            # fusion head
            for g in range(2):
                nc.vector.memset(racc[g], 0.0)
            for c0, c1 in CH:
                p_1 = pmm.tile([128, 1024], F32, tag="pdl", name="p_1")
                mm512(p_1, W_lin1a, xm_hat, c0, c1, start=True, stop=False)
                mm512(p_1, W_lin1b, xcnn, c0, c1, start=False, stop=True)
                mneg = ws.tile([128, 1024], BF16, tag="mneg", name="mneg")
                nc.vector.tensor_scalar(mneg, p_1, Vec[V_BHEAD1], 0.0,
                                        OP.add, OP.min)
                e_t = ws.tile([128, 1024], BF16, tag="e_t", name="e_t")
                act(e_t, mneg, AF.Exp)
                r_t = ws.tile([128, 1024], BF16, tag="r_t", name="r_t")
                act(r_t, p_1, AF.Relu, bias=Vec[V_BHEAD1])
                v_t = ws.tile([128, 1024], BF16, tag="e_t", name="v_t")
                nc.vector.tensor_add(v_t, r_t, e_t)
                for g in range(2):
                    p_o2 = pmm.tile([128, 1024], F32, tag="pdx", name="p_o2")
                    mm512(p_o2, W_lin2[g], v_t, 0, 1024)
                    o2c = ws.tile([128, 1024], BF16, tag="mneg", name="o2c")
                    act(o2c, p_o2, AF.Identity, bias=Vec[V_BLIN2])
                    p_o3 = pmm.tile([4, 1024], F32, tag="pdl", name="p_o3")
                    mm512(p_o3, W_lin3[g], o2c, 0, 1024)
                    o3c = ws.tile([4, 1024], F32, tag="sd", name="o3c")
                    act(o3c, p_o3, AF.Copy)
                    rc = ws.tile([4, 1], F32, tag="rc", name="rc")
                    nc.vector.tensor_reduce(rc, o3c, AX.X, OP.add)
                    nc.vector.tensor_add(racc[g], racc[g], rc)
            for g in range(2):
                res = sg.tile([4, 1], F32, tag=f"res{g}", name=f"res{g}")
                act(res, racc[g], AF.Sigmoid, bias=Bout[0:4, :],
                    scale=1.0 / N)
                nc.sync.dma_start(out=out_d[4 * g:4 * g + 4, :], in_=res)

        # ================= phase A: fused in_proj + causal conv, silu =======
        for g in range(2):
            xsp = [ws.tile([96, N], BF16, tag="pairA", name="xsp")
                   for j in range(2)]
            for j in range(2):
                for ph in range(2):
                    nc.sync.dma_start(out=xsp[j][48 * ph:48 * ph + 48, :],
                                      in_=xs_d[2 * g + j, 48 * ph:48 * ph + 48])
            for c0 in range(0, N, 1024):
                sl = slice(c0, c0 + 1024)
                for j in range(2):
                    jj = slice(64 * j, 64 * j + 64)
                    p_xc = pmm.tile([64, 1024], F32, tag="pdl", name="pmm")
                    for o in (0, 512):
                        nc.tensor.matmul(p_xc[:, o:o + 512], W_xc,
                                         xsp[j][:, c0 + o:c0 + o + 512])
                    act(xi[g][jj, sl], p_xc, AF.Silu,
                        bias=Vec[V_BCONV][jj, :])
                    p_z = pmm.tile([64, 1024], F32, tag="pdx", name="pmm")
                    for o in (0, 512):
                        nc.tensor.matmul(p_z[:, o:o + 512], W_z,
                                         xsp[j][:, c0 + o:c0 + o + 512])
                    act(siluz[g][jj, sl], p_z, AF.Silu)

        # ================= phase B: x_proj (delta folded), dx ==============
        for g in range(2):
            for c0 in range(0, N, 1024):
                sl = slice(c0, c0 + 1024)
                p_d = pmm.tile([128, 1024], F32, tag="pdl", name="pmm")
                p_bc = pmm.tile([128, 1024], F32, tag="pdx", name="pmm")
                for o in (0, 512):
                    nc.tensor.matmul(p_d[:, o:o + 512], W_delta,
                                     xi[g][:, c0 + o:c0 + o + 512])
                    nc.tensor.matmul(p_bc[:, o:o + 512], W_bc,
                                     xi[g][:, c0 + o:c0 + o + 512])
                edt = ws.tile([128, 1024], F32, tag="sgm", name="edt")
                act(edt, p_d, AF.Exp, bias=Vec[V_BDT])
                act(delta[g][:, sl], edt, AF.Ln, bias=1.0)
                act(BC[g][:, sl], p_bc, AF.Copy)
            nc.vector.tensor_mul(dx[g], delta[g], xi[g])

        # ================= phase C: selective scan ==========================
        for b in range(BL):
            g, bi = b // 4, b % 4
            # B/C broadcast via mask-matmul + copy: lane 8*n+dl <- row r
            Bbc = cp.tile([128, N], BF16, tag="Bbc", name="Bbc")
            Cbc = cp.tile([128, N], BF16, tag="Cbc", name="Cbc")
            for (mask, dst), tg in (((M_b[bi], Bbc), "pdl"),
                                    ((M_c[bi], Cbc), "pdx")):
                for c0 in range(0, N, 1024):
                    p_b = pmm.tile([128, 1024], F32, tag=tg, name="pbc")
                    for o in (0, 512):
                        nc.tensor.matmul(p_b[:, o:o + 512], mask,
                                         BC[g][:, c0 + o:c0 + o + 512])
                    act(dst[:, c0:c0 + 1024], p_b, AF.Copy)
            hcs = []
            for t in range(3):
                a_t = cp.tile([128, N], BF16, tag="a_t", name="a_t")
                dBx = cp.tile([128, N], BF16, tag="dBx", name="dBx")
                for c0 in range(0, N, 1024):
                    sl = slice(c0, c0 + 1024)
                    p_dl = pmm.tile([128, 1024], F32, tag="pdl", name="p_dl")
                    p_dx = pmm.tile([128, 1024], F32, tag="pdx", name="p_dx")
                    for o in (0, 512):
                        nc.tensor.matmul(p_dl[:, o:o + 512], M_dbc[3 * bi + t],
                                         delta[g][:, c0 + o:c0 + o + 512])
                        nc.tensor.matmul(p_dx[:, o:o + 512], M_dbc[3 * bi + t],
                                         dx[g][:, c0 + o:c0 + o + 512])
                    act(a_t[:, sl], p_dl, AF.Exp, scale=ScA[t])
                    nc.vector.tensor_mul(dBx[:, sl], p_dx, Bbc[:, sl])
                h_t = cp.tile([128, N], BF16, tag="h_t", name="h_t", bufs=1)
                nc.vector.tensor_tensor_scan(h_t, a_t, dBx, 0.0,
                                             OP.mult, OP.add)
                hc = hcp.tile([128, N], BF16, tag=f"hc{t}", name="hc")
                heng = nc.vector
                for c0 in C4:
                    heng.tensor_mul(hc[:, c0:c0 + NC2], h_t[:, c0:c0 + NC2],
                                    Cbc[:, c0:c0 + NC2])
                hcs.append(hc)
            # y = (ys + xi*Dp) * silu(z)
            rr = slice(32 * bi, 32 * bi + 32)
            for c0 in C4:
                sl = slice(c0, c0 + NC2)
                p_yt = pd.tile([32, NC2], F32, tag="pd", name="pyt")
                for t in range(3):
                    nc.tensor.matmul(p_yt, W_mask[t], hcs[t][:, sl],
                                     start=(t == 0), stop=False)
                nc.tensor.matmul(p_yt, W_dp[bi], xi[g][:, sl],
                                 start=False, stop=True)
                nc.vector.tensor_mul(y[g][rr, sl], p_yt, siluz[g][rr, sl])

        _phase_E()
        _phase_DF()

    # Prefer the combined ln+exp ACT table: hide Exp/Ln from all other
    # tables so the table-load pass lands on natural_log_exp_and_others
    # (availability-only metadata; claiming less than reality is safe).
    import concourse.bacc as bacc_mod
    from concourse import mybir as _mb
    _orig_gat = bacc_mod.get_activation_tables

    def _gat(arch):
        t = {k: set(v) for k, v in _orig_gat(arch).items()}
        for name, s in t.items():
            if name != "natural_log_exp_and_others":
                s.discard(_mb.ActivationFunctionType.Exp)
                s.discard(_mb.ActivationFunctionType.Ln)
        return t

    bacc_mod.get_activation_tables = _gat
    try:
        nc.compile()
    finally:
        bacc_mod.get_activation_tables = _orig_gat
    return nc


# ---------------------------------------------------------------- host side
def _host_prep(inputs):
    f32, f16 = np.float32, np.float16
    x = inputs["x"].astype(f32)
    in_proj_w = inputs["in_proj_w"].astype(f32)
    conv_w = inputs["conv_w"].astype(f32)
    conv_b = inputs["conv_b"].astype(f32)
    x_proj_w = inputs["x_proj_w"].astype(f32)
    dt_w = inputs["dt_w"].astype(f32)
    dt_b = inputs["dt_b"].astype(f32)
    A_log = inputs["A_log"].astype(f32)
    Dp = inputs["Dp"].astype(f32)
    out_proj_w = inputs["out_proj_w"].astype(f32)
    ln1_g, ln1_b = inputs["ln1_g"].astype(f32), inputs["ln1_b"].astype(f32)
    ffn_w1, ffn_b1 = inputs["ffn_w1"].astype(f32), inputs["ffn_b1"].astype(f32)
    ffn_w2, ffn_b2 = inputs["ffn_w2"].astype(f32), inputs["ffn_b2"].astype(f32)
    ffn_ln_g = inputs["ffn_ln_g"].astype(f32)
    ffn_ln_b = inputs["ffn_ln_b"].astype(f32)
    cnn_w, cnn_b = inputs["cnn_w"].astype(f32), inputs["cnn_b"].astype(f32)
    lin1_w, lin1_b = inputs["lin1_w"].astype(f32), inputs["lin1_b"].astype(f32)
    lin2_w, lin2_b = inputs["lin2_w"].astype(f32), inputs["lin2_b"].astype(f32)
    lin3_w, lin3_b = inputs["lin3_w"].astype(f32), inputs["lin3_b"].astype(f32)

    sh = {}
    # fused in_proj + conv:  Wxc[k*12+m, d] = conv_w[d,0,k]*in_proj_w[d,m]
    Wxc = np.einsum('dk,dm->kmd', conv_w[:, 0, :], in_proj_w[:DI]).reshape(48, DI)
    sh["w_xc"] = np.zeros((96, 64), f32)
    sh["w_z"] = np.zeros((96, 64), f32)
    for b2 in range(2):
        sh["w_xc"][48 * b2:48 * b2 + 48, 32 * b2:32 * b2 + 24] = Wxc
        for m in range(DM):
            sh["w_z"][48 * b2 + 36 + m, 32 * b2:32 * b2 + 24] = in_proj_w[DI:, m]
    # x_proj (delta rank-1 folded)
    Wdelta = np.einsum('d,j->jd', dt_w[:, 0], x_proj_w[0])     # [24,24]
    WBC = x_proj_w[1:].T                                       # [24,32]
    sh["w_delta"] = np.zeros((128, 128), f32)
    sh["w_bc"] = np.zeros((128, 128), f32)
    for bi in range(4):
        r = slice(32 * bi, 32 * bi + 24)
        sh["w_delta"][r, 32 * bi:32 * bi + 24] = Wdelta
        sh["w_bc"][r, 32 * bi:32 * bi + 32] = WBC
    # out_proj with centering fold
    Pc = np.eye(DM, dtype=f32) - f32(1.0 / DM)
    WopT = (Pc @ out_proj_w).T                                 # [24,12]
    sh["w_op"] = np.zeros((128, 64), f32)
    for bi in range(4):
        sh["w_op"][32 * bi:32 * bi + 24, 16 * bi:16 * bi + 12] = WopT
    sh["w_ones12"] = np.zeros((128, 8), f32)
    sh["w_bc8"] = np.zeros((8, 128), f32)
    for b in range(8):
        sh["w_ones12"][16 * b:16 * b + 12, b] = f32(1.0 / DM)
        sh["w_bc8"][b, 16 * b:16 * b + 16] = 1.0
    # ffn (0.5 of exact-gelu folded into w_ffn2)
    W1p = (ffn_w1 * ln1_g[None, :]).T                          # [12,48]
    b1p = ffn_b1 + ffn_w1 @ ln1_b
    sh["w_ffn1"] = np.zeros((4, 128, 128), f32)
    sh["w_ffn2"] = np.zeros((4, 128, 32), f32)
    for q in range(4):
        for b2 in range(2):
            b = 2 * q + b2
            sh["w_ffn1"][q, 16 * b:16 * b + 12, 64 * b2:64 * b2 + 48] = W1p
            sh["w_ffn2"][q, 64 * b2:64 * b2 + 48,
                         16 * b2:16 * b2 + 12] = 0.5 * ffn_w2.T
    sh["w_pc"] = np.zeros((128, 128), f32)
    W1aT = (lin1_w[:, :DM] * ffn_ln_g[None, :]).T              # [12,12]
    W1bT = lin1_w[:, DM:].T
    sh["w_lin1a"] = np.zeros((128, 128), f32)
    sh["w_lin1b"] = np.zeros((128, 128), f32)
    for b in range(8):
        r = slice(16 * b, 16 * b + 12)
        sh["w_pc"][r, r] = Pc
        sh["w_lin1a"][r, r] = W1aT
        sh["w_lin1b"][r, r] = W1bT
    b1h = lin1_b + lin1_w[:, :DM] @ ffn_ln_b
    b2p = lin2_b - lin2_w.sum(axis=1)
    sh["w_lin2"] = np.zeros((2, 128, 128), f32)
    sh["w_lin3"] = np.zeros((2, 128, 4), f32)
    for g in range(2):
        for bp in range(4):
            b = 4 * g + bp
            sh["w_lin2"][g, 16 * b:16 * b + 12,
                         32 * bp:32 * bp + 20] = lin2_w.T
            sh["w_lin3"][g, 32 * bp:32 * bp + 20, bp] = lin3_w[0]
    sh["w_cnn"] = np.zeros((3, 96, 128), f16)
    for k in range(3):
        for b in range(8):
            sh["w_cnn"][k, 12 * b:12 * b + 12,
                        16 * b:16 * b + 12] = cnn_w[:, :, k].T.astype(f16)
    # scan masks and A scales (lane = 8*n + dl)
    sh["w_mask"] = np.zeros((3, 128, 32), np.float32)
    sh["sc_negA"] = np.zeros((3, 128, 1), f32)
    Asc = -np.exp(A_log)                                       # [24,16]
    for t in range(3):
        for dl in range(8):
            for n in range(DS):
                sh["w_mask"][t, 8 * n + dl, 8 * t + dl] = 1.0
                sh["sc_negA"][t, 8 * n + dl, 0] = Asc[8 * t + dl, n]
    # broadcast masks: 12 (bi,t3) for delta/dx rows + 4 B + 4 C
    masks = np.zeros((128, 20 * 128), f32)
    for bi in range(4):
        for t in range(3):
            i = 3 * bi + t
            for dl in range(8):
                for n in range(DS):
                    masks[32 * bi + 8 * t + dl, 128 * i + 8 * n + dl] = 1.0
        for dl in range(8):
            for n in range(DS):
                masks[32 * bi + n, 128 * (12 + bi) + 8 * n + dl] = 1.0
                masks[32 * bi + 16 + n, 128 * (16 + bi) + 8 * n + dl] = 1.0
    sh["masks"] = masks

    def pack(v, blk, nblk):
        o = np.zeros(128, f32)
        for i in range(nblk):
            o[blk * i:blk * i + len(v)] = v
        return o

    vecs = np.zeros((128, 11), f32)
    bconv64 = np.zeros(64, f32)
    bconv64[0:24] = conv_b
    bconv64[32:56] = conv_b
    vecs[:, 0] = np.concatenate([bconv64, bconv64])
    vecs[:, 1] = pack(dt_b, 32, 4)
    vecs[:, 2] = pack(Dp, 32, 4)
    vecs[:, 3] = pack(ln1_g, 16, 8)
    vecs[:, 4] = pack(ln1_b, 16, 8)
    vecs[:, 5] = pack(b1p, 64, 2)
    vecs[:, 6] = pack(ffn_b2, 16, 8)
    vecs[:, 7] = pack(b1h, 16, 8)
    vecs[:, 8] = pack(b2p, 32, 4)
    vecs[:, 9] = pack(cnn_b, 16, 8)
    vecs[:, 10] = pack(b1p * f32(SQ2I), 64, 2)
    sh["vecs"] = vecs
    sh["w_dp"] = np.zeros((4, 128, 32), f32)
    for bi in range(4):
        for c in range(DI):
            sh["w_dp"][bi, 32 * bi + c, c] = Dp[c]
    sh["b_out"] = np.full((8, 1), lin3_b[0], f32)
    sh["b_eps"] = np.full((8, 1), 1e-12, f32)
    # DFT matrices as moving operand: wdft[cs, kt, t1, f]
    t_ = np.arange(L, dtype=np.float64)
    f_ = np.arange(NF, dtype=np.float64)
    ang = (2 * np.pi / L) * np.outer(t_, f_)                   # [t, f]
    wc = np.cos(ang)
    wsn = np.sin(ang)
    wc[:, 1025:] = 0.0
    wsn[:, 1025:] = 0.0
    wdft = np.zeros((2, NKT, 128, 1024), f16)
    for ch in range(2):
        f0 = 512 * ch
        for kt in range(NKT):
            rows = slice(128 * kt, 128 * kt + 128)
            wdft[ch, kt, :, 0:512] = wc[rows, f0:f0 + 512].astype(f16)
            wdft[ch, kt, :, 512:1024] = wsn[rows, f0:f0 + 512].astype(f16)
    sh["wdft"] = wdft
    nyc = np.zeros((128, NKT), f16)
    for kt in range(NKT):
        nyc[:, kt] = wc[128 * kt:128 * kt + 128, 1024].astype(f16)
    sh["nyc"] = nyc

    import ml_dtypes
    for k in ("w_mask", "w_xc", "w_z", "w_delta", "w_bc", "w_dp", "w_op",
              "w_ffn1", "w_ffn2", "w_pc", "w_lin1a", "w_lin1b", "w_lin2",
              "w_lin3", "masks", "w_ones12"):
        sh[k] = sh[k].astype(ml_dtypes.bfloat16)

    # per-core data
    per_core = []
    for c in range(NCORES):
        xl = x[BL * c:BL * c + BL]                             # [8,2048,12]
        xs = np.zeros((4, 96, N), f32)
        for j in range(4):
            for b2 in range(2):
                xb = xl[2 * j + b2]                            # [2048,12]
                for k in range(4):
                    shf = 3 - k
                    r0 = 48 * b2 + 12 * k
                    if shf == 0:
                        xs[j, r0:r0 + 12, :] = xb.T
                    else:
                        xs[j, r0:r0 + 12, shf:] = xb[:-shf].T
        xt = np.zeros((128, NKT * 96), f16)
        for kt in range(NKT):
            xt[:, 96 * kt:96 * kt + 96] = \
                xl[:, 128 * kt:128 * kt + 128].transpose(1, 0, 2) \
                .reshape(128, 96).astype(f16)
        per_core.append({"xs": xs.astype(ml_dtypes.bfloat16), "xt": xt})
    return sh, per_core


def kernel(**inputs):
    sh, per_core = _host_prep(inputs)
    if "nc" not in _CACHE:
        _CACHE["nc"] = _build_module()
    nc = _CACHE["nc"]
    in_maps = [{**sh, **pc} for pc in per_core]
    from concourse.bass_utils import run_bass_kernel_spmd
    res = run_bass_kernel_spmd(nc, in_maps, core_ids=list(range(NCORES)))
    outs = [res.results[c]["out"].reshape(BL) for c in range(NCORES)]
    return np.concatenate(outs).astype(np.float32)
